# revision 20
# baseline (speedup 1.0000x reference)
"""Trainium2 Bass kernel v2 for the 2-layer heterogeneous GAT (drug/cell).

Strategy (8 NeuronCores, SPMD single program), changes vs v1:
  - All five edge phases partitioned by DST block; dc2 now dst-partitioned
    too, fed by an AllGather of hd1 (bf16) instead of AllToAll partials.
  - bf16 gather tables, 384-col rows (768B, %256B) holding [fs 256 | el 8].
  - er is never gathered: per-dst-tile er vectors [128, 8] are stashed in
    SBUF (projected from local features / hc1 tiles) and broadcast to edges
    with a one-hot matmul (lhsT = S2[dst -> edge]).
  - Edge phase batches DVE work per group of <=16 chunks: one is_equal for
    all S chunks, one for S2 (dst-row stream replicated via DMA broadcast),
    one fused exp/leaky pipeline, one rhs build.
  - Gathers up to 1024 idx per call (SWDGE ring limit is < 2048).
  - Projections batched: 8 tiles per DMA load/store, psum->SBUF copies
    alternate ACT/DVE.
"""
import sys
sys.path.insert(0, '/opt/trn_rl_repo')
import numpy as np
import ml_dtypes

import concourse.bacc as bacc
import concourse.tile as tile
from concourse import mybir, library_config

F32 = mybir.dt.float32
BF16 = mybir.dt.bfloat16
FP8 = mybir.dt.float8e4
I16 = mybir.dt.int16
BF = ml_dtypes.bfloat16
F8 = mybir.dt.np(FP8)
P = 128
H = 8
FD = 256          # feature dim
TW = 384          # gather-table row width (bf16) = 768B; payload 264
GSZ = 16          # chunks per batched edge group
GCH = 8           # chunks per dma_gather call (1024 idx; 2048 hangs)
AluOp = mybir.AluOpType
Act = mybir.ActivationFunctionType

N_CORES = 8


def legalize_waits(nc):
    """Split multi-wait instructions into wait-carrying NOP chains."""
    n_split = 0
    for fn in nc.m.functions:
        for bb in fn.blocks:
            insts = bb.instructions
            new = []
            changed = False
            for inst in insts:
                si = inst.sync_info
                waits = list(si.on_wait) if si is not None else []
                cap = 2 if isinstance(inst, mybir.InstEventSemaphore) else 1
                if len(waits) > cap:
                    keep = waits[-cap:]
                    for w in waits[:-cap]:
                        nop = mybir.InstNoOp(
                            name=nc.get_next_instruction_name(),
                            engine=inst.engine,
                            sync_info=mybir.SyncInfo(on_wait=[w], on_update=[]),
                            bass_nofuse=True,
                        )
                        new.append(nop)
                        n_split += 1
                    inst.sync_info = mybir.SyncInfo(
                        on_wait=keep, on_update=list(si.on_update))
                    changed = True
                new.append(inst)
            if changed:
                bb.instructions = new
    return n_split


def make_cfg(Nd, Nc):
    ndp = -(-Nd // 1024) * 1024
    ncp = -(-Nc // 1024) * 1024
    return dict(Nd=Nd, Nc=Nc, NDP=ndp, NCP=ncp, DBLK=ndp // 8,
                CBLK=ncp // 8, HALF=ndp // 2, n_cores=8)


# --------------------------------------------------------------------------
# host-side prep
# --------------------------------------------------------------------------

def _fold_weights(ip):
    def wel(W, a):  # W [256,256], a [H,D] -> [256,H]
        return (np.asarray(W, np.float32).reshape(FD, H, -1)
                * np.asarray(a, np.float32)[None]).sum(-1)
    Wsrc = np.asarray(ip['Wsrc'], np.float32)
    Wdst = np.asarray(ip['Wdst_dc'], np.float32)
    al = np.asarray(ip['attn_l'], np.float32)
    ar = np.asarray(ip['attn_r'], np.float32)

    def w264(W, a):
        return np.concatenate([W, wel(W, a)], 1)  # [256, 264]

    def bftile(a, ncols):  # [256, ncols] -> [2, 128, ncols] bf16
        return np.ascontiguousarray(a.reshape(2, P, ncols)).astype(BF)

    out = {}
    out['WDD'] = bftile(w264(Wsrc[0, 0], al[0, 0]), 264)
    out['WDC'] = bftile(w264(Wsrc[0, 1], al[0, 1]), 264)
    out['WCC1'] = bftile(w264(Wsrc[0, 2], al[0, 2]), 264)
    out['WDC2'] = bftile(w264(Wsrc[1, 1], al[1, 1]), 264)
    out['WCC2'] = bftile(w264(Wsrc[1, 2], al[1, 2]), 264)
    out['WerDD'] = bftile(wel(Wsrc[0, 0], ar[0, 0]), 8)
    out['WerC1'] = bftile(np.concatenate(
        [wel(Wdst[0], ar[0, 1]), wel(Wsrc[0, 2], ar[0, 2])], 1), 16)
    out['WerC2'] = bftile(np.concatenate(
        [wel(Wdst[1], ar[1, 1]), wel(Wsrc[1, 2], ar[1, 2])], 1), 16)
    out['bias5'] = np.stack([ip['gat_bias'][0, 0], ip['gat_bias'][0, 1],
                             ip['gat_bias'][0, 2], ip['gat_bias'][1, 1],
                             ip['gat_bias'][1, 2]]).astype(np.float32)
    out['semW1'] = np.stack([np.asarray(ip['sem_W1'][l, 1], np.float32)
                             .reshape(2, P, P) for l in range(2)]).astype(BF)
    out['semb1'] = np.stack([np.asarray(ip['sem_b1'][l, 1], np.float32)
                             .reshape(P, 1) for l in range(2)]).astype(np.float32)
    out['semW2'] = np.stack([np.asarray(ip['sem_W2'][l, 1], np.float32)
                             .reshape(P, 1) for l in range(2)]).astype(BF)
    out['dnnW1'] = np.asarray(ip['dnn_W1'], np.float32).reshape(2, P, 32).astype(BF)
    out['dnnb1'] = np.asarray(ip['dnn_b1'], np.float32).reshape(32, 1)
    out['dnnW2'] = np.asarray(ip['dnn_W2'], np.float32).astype(BF)
    out['dnnb2'] = np.asarray(ip['dnn_b2'], np.float32).reshape(16, 1)
    out['dnnW3'] = np.asarray(ip['dnn_W3'], np.float32).astype(BF)
    out['dnnb3'] = np.asarray(ip['dnn_b3'], np.float32).reshape(1, 1)
    return out


def _wrap16(vals):
    n = len(vals)
    out = np.zeros((P, n // 16), np.int16)
    a = np.asarray(vals, np.int16).reshape(-1, 16).T
    for g in range(8):
        out[g * 16:(g + 1) * 16, :] = a
    return out


def _prep_rel(src, dst, nco, *, blk, n_tiles, halves):
    """Per-core edge schedule, dst-partitioned.

    Returns nch [T, n_halves] (equalized over cores) and per-core streams:
    src idx wrapped i16 [P, tot*8], plus host-built one-hot scatter
    matrices in fp8: S [P, tot*128] (S[p, k*128+d] = dl[p,k]==d) and
    S2 [P, tot*128] (S2[p, k*128+e] = dl[e,k]==p).
    """
    src = np.asarray(src, np.int64)
    dst = np.asarray(dst, np.int64)
    per_core = []
    for c in range(nco):
        m = (dst >= c * blk) & (dst < (c + 1) * blk)
        dl = dst[m] - c * blk
        s = src[m]
        tid = dl // P
        per_core.append((s, dl % P, tid))
    nh = 2 if halves else 1
    nch = np.zeros((n_tiles, nh), np.int64)
    buckets = []
    for c in range(nco):
        s, dloc, tid = per_core[c]
        bk = {}
        for t in range(n_tiles):
            mt = tid == t
            st, dt_ = s[mt], dloc[mt]
            if halves:
                m0 = st < halves
                groups = [(st[m0], dt_[m0]), (st[~m0] - halves, dt_[~m0])]
            else:
                groups = [(st, dt_)]
            bk[t] = groups
            for h, (gs, gd) in enumerate(groups):
                nch[t, h] = max(nch[t, h], (len(gs) + P - 1) // P)
        buckets.append(bk)
    nch = np.maximum(nch, 1)
    tot = int(nch.sum())
    rng = np.arange(P, dtype=np.int64)
    srcs, Ss, S2s = [], [], []
    for c in range(nco):
        bk = buckets[c]
        sw = np.zeros((P, tot * 8), np.int16)
        dall = np.full((tot, P), -1, np.int64)   # [chunk, edge] local dst
        col = 0
        for t in range(n_tiles):
            for h, (gs, gd) in enumerate(bk[t]):
                n = int(nch[t, h]) * P
                a = np.zeros(n, np.int64)
                a[:len(gs)] = gs
                assert a.max(initial=0) < 32768
                sw[:, col * 8:(col + int(nch[t, h])) * 8] = _wrap16(a)
                dpad = np.full(n, -1, np.int64)
                dpad[:len(gd)] = gd
                dall[col:col + int(nch[t, h])] = dpad.reshape(-1, P)
                col += int(nch[t, h])
        # S[p, k, d] = (dall[k, p] == d);  S2[p, k, e] = (dall[k, e] == p)
        S = (dall.T[:, :, None] == rng[None, None, :]).astype(F8)
        S2 = (dall[None, :, :] == rng[:, None, None]).astype(F8)
        srcs.append(sw)
        Ss.append(np.ascontiguousarray(S.reshape(P, tot * P)))
        S2s.append(np.ascontiguousarray(S2.reshape(P, tot * P)))
    return dict(nch=nch, src=srcs, S=Ss, S2=S2s, tot=tot)


def host_prep(ip, cfg):
    W = _fold_weights(ip)
    nco = cfg['n_cores']
    DBLK, CBLK, HALF = cfg['DBLK'], cfg['CBLK'], cfg['HALF']
    NDP, NCP = cfg['NDP'], cfg['NCP']
    DD_T, CT = DBLK // P, CBLK // P

    dd = _prep_rel(ip['src_dd'], ip['dst_dd'], nco, blk=DBLK,
                   n_tiles=DD_T, halves=HALF)
    dc = _prep_rel(ip['src_dc'], ip['dst_dc'], nco, blk=CBLK,
                   n_tiles=CT, halves=HALF)
    cc = _prep_rel(ip['src_cc'], ip['dst_cc'], nco, blk=CBLK,
                   n_tiles=CT, halves=None)

    featD = np.zeros((NDP, FD), np.float32)
    featD[:cfg['Nd']] = np.asarray(ip['feat_drug'], np.float32)
    featC = np.zeros((NCP, FD), np.float32)
    featC[:cfg['Nc']] = np.asarray(ip['feat_cell'], np.float32)
    featDT = np.ascontiguousarray(featD.T).reshape(2, P, NDP).astype(BF)
    featCT = np.ascontiguousarray(featC.T).reshape(2, P, NCP).astype(BF)

    identB = np.eye(P, dtype=np.float32).astype(BF)

    base = dict(featDT=featDT, featCT=featCT, identB=identB, **W)
    in_maps = []
    for c in range(nco):
        m = dict(base)
        m['featDTloc'] = np.ascontiguousarray(
            featD[c * DBLK:(c + 1) * DBLK].T).reshape(2, P, DBLK).astype(BF)
        m['featCTloc'] = np.ascontiguousarray(
            featC[c * CBLK:(c + 1) * CBLK].T).reshape(2, P, CBLK).astype(BF)
        for r, d in (('dd', dd), ('dc', dc), ('cc', cc)):
            m[f'{r}_src'] = d['src'][c]
            m[f'{r}_S'] = d['S'][c]
            m[f'{r}_S2'] = d['S2'][c]
        in_maps.append(m)
    sched = dict(dd=dd['nch'], dc=dc['nch'], cc=cc['nch'])
    return sched, in_maps


# --------------------------------------------------------------------------
# device program
# --------------------------------------------------------------------------

def _tile_segments(nch_row):
    """Per-tile gather segments [(start_col, n, half)], respecting half
    boundaries, GSZ group windows, and GCH call caps."""
    bounds = []
    off = 0
    for h, nh in enumerate(nch_row):
        bounds.append((off, off + int(nh), h))
        off += int(nh)
    tnch = off
    segs = []
    for g0 in range(0, tnch, GSZ):
        g1 = min(g0 + GSZ, tnch)
        for (h0, h1, h) in bounds:
            s0 = max(g0, h0)
            s1 = min(g1, h1)
            for b in range(s0, s1, GCH):
                segs.append((b, min(GCH, s1 - b), h))
    return segs, tnch


def build_program(sched, cfg, legalize=True):
    nco = cfg['n_cores']
    DBLK, CBLK, HALF = cfg['DBLK'], cfg['CBLK'], cfg['HALF']
    NDP, NCP = cfg['NDP'], cfg['NCP']
    DD_T, CT = DBLK // P, CBLK // P
    NDT, NCT = NDP // P, NCP // P

    nc = bacc.Bacc(None)
    d = {}
    def inp(name, shape, dt=BF16):
        d[name] = nc.declare_dram_parameter(name, list(shape), dt,
                                            isOutput=False)
        return d[name]

    featDT = inp('featDT', (2, P, NDP))
    featCT = inp('featCT', (2, P, NCP))
    featDTloc = inp('featDTloc', (2, P, DBLK))
    featCTloc = inp('featCTloc', (2, P, CBLK))
    identB_in = inp('identB', (P, P))
    Wmain = {k: inp(k, (2, P, 264)) for k in
             ('WDD', 'WDC', 'WCC1', 'WDC2', 'WCC2')}
    WerDD = inp('WerDD', (2, P, 8))
    WerC1 = inp('WerC1', (2, P, 16))
    WerC2 = inp('WerC2', (2, P, 16))
    bias5 = inp('bias5', (5, FD), F32)
    semW1 = inp('semW1', (2, 2, P, P))
    semb1 = inp('semb1', (2, P, 1), F32)
    semW2 = inp('semW2', (2, P, 1))
    dnnW1 = inp('dnnW1', (2, P, 32)); dnnb1 = inp('dnnb1', (32, 1), F32)
    dnnW2 = inp('dnnW2', (32, 16)); dnnb2 = inp('dnnb2', (16, 1), F32)
    dnnW3 = inp('dnnW3', (16, 1)); dnnb3 = inp('dnnb3', (1, 1), F32)
    streams = {}
    for r, nchs in sched.items():
        tot = int(nchs.sum())
        streams[r] = (inp(f'{r}_src', (P, tot * 8), I16),
                      inp(f'{r}_S', (P, tot * P), FP8),
                      inp(f'{r}_S2', (P, tot * P), FP8))
    out = nc.declare_dram_parameter('out', [1, CBLK], F32, isOutput=True)

    with tile.TileContext(nc) as tc:
        with tc.tile_pool(name="const", bufs=1) as cpool, \
             tc.tile_pool(name="sb", bufs=3) as sb, \
             tc.tile_pool(name="pl", bufs=3) as pl, \
             tc.tile_pool(name="gb", bufs=3) as gb, \
             tc.tile_pool(name="eb", bufs=3) as eb, \
             tc.tile_pool(name="ps", bufs=2, space="PSUM") as ps, \
             tc.tile_pool(name="psm", bufs=2, space="PSUM") as psm, \
             tc.tile_pool(name="dram", bufs=1, space="DRAM") as dr:
            nc.gpsimd.load_library(library_config.mlp)

            # ---- constants in SBUF
            C = {}
            def cload(name, ap, shape, dt=BF16):
                t_ = cpool.tile(list(shape), dt, tag=f"c_{name}")
                nc.sync.dma_start(t_[:], ap)
                return t_
            C['ident'] = cload('ident', identB_in[:], (P, P))
            ones1 = cpool.tile([1, P], BF16)
            nc.vector.memset(ones1[:], 1.0)
            wt = {}
            for k in Wmain:
                wt[k] = tuple(cload(f'{k}{j}', Wmain[k][j], (P, 264))
                              for j in range(2))
            werDD = tuple(cload(f'werDD{j}', WerDD[j], (P, 8))
                          for j in range(2))
            werC1 = tuple(cload(f'werC1{j}', WerC1[j], (P, 16))
                          for j in range(2))
            werC2 = tuple(cload(f'werC2{j}', WerC2[j], (P, 16))
                          for j in range(2))
            btiles = [cload(f'bias{r}', bias5[r:r + 1, :].to_broadcast([P, FD]),
                            (P, FD), F32) for r in range(5)]
            sw1 = [tuple(cload(f'sw1_{l}{j}', semW1[l, j], (P, P))
                         for j in range(2)) for l in range(2)]
            sb1 = [cload(f'sb1_{l}', semb1[l], (P, 1), F32) for l in range(2)]
            sw2 = [cload(f'sw2_{l}', semW2[l], (P, 1)) for l in range(2)]
            dW1 = tuple(cload(f'dW1{j}', dnnW1[j], (P, 32)) for j in range(2))
            dW2 = cload('dW2', dnnW2[:], (32, 16))
            dW3 = cload('dW3', dnnW3[:], (16, 1))
            db1 = cload('db1', dnnb1[:], (32, 1), F32)
            db2 = cload('db2', dnnb2[:], (16, 1), F32)
            db3 = cload('db3', dnnb3[:], (1, 1), F32)
            # resident local features (for er projections)
            fdl = tuple(cload(f'fdl{j}', featDTloc[j], (P, DBLK))
                        for j in range(2))
            fcl = tuple(cload(f'fcl{j}', featCTloc[j], (P, CBLK))
                        for j in range(2))
            # resident edge streams
            SI = {}
            for r in sched:
                tot = int(sched[r].sum())
                SI[r] = cload(f'si_{r}', streams[r][0][:], (P, tot * 8), I16)
            # er stashes
            erDDs = cpool.tile([P, DD_T * 8], BF16, tag="erDDs")
            erC1s = cpool.tile([P, CT * 16], BF16, tag="erC1s")
            erC2s = cpool.tile([P, CT * 16], BF16, tag="erC2s")

            # ---- internal DRAM
            tabDD = dr.tile([NDP, TW], BF16)
            tabDC = dr.tile([NDP, TW], BF16)
            tabCC1 = dr.tile([NCP, TW], BF16)
            tabDC2loc = dr.tile([DBLK, TW], BF16)
            tabDC2 = dr.tile([NDP, TW], BF16, addr_space="Shared")
            tabCC2loc = dr.tile([CBLK, TW], BF16)
            tabCC2 = dr.tile([NCP, TW], BF16, addr_space="Shared")
            hd1T = dr.tile([2 * P, DBLK], BF16)
            oDC1T = dr.tile([2 * P, CBLK], BF16)
            oCC1T = dr.tile([2 * P, CBLK], BF16)
            oDC2T = dr.tile([2 * P, CBLK], BF16)
            oCC2T = dr.tile([2 * P, CBLK], BF16)
            hc1T = dr.tile([2 * P, CBLK], BF16)

            # ---- er stash projections (local features)
            def er_stash(fres, wpair, stash, n_tiles, ncols):
                BT = min(8, n_tiles) if ncols == 8 else n_tiles
                for t0 in range(0, n_tiles, BT):
                    bt = min(BT, n_tiles - t0)
                    pp = ps.tile([P, bt * ncols], F32, tag="projps")
                    for i in range(bt):
                        tl = (t0 + i) * P
                        nc.tensor.matmul(pp[:, i * ncols:(i + 1) * ncols],
                                         lhsT=fres[0][:, tl:tl + P],
                                         rhs=wpair[0][:], start=True, stop=False)
                        nc.tensor.matmul(pp[:, i * ncols:(i + 1) * ncols],
                                         lhsT=fres[1][:, tl:tl + P],
                                         rhs=wpair[1][:], start=False, stop=True)
                    nc.scalar.copy(stash[:, t0 * ncols:(t0 + bt) * ncols], pp[:])
            er_stash(fdl, werDD, erDDs, DD_T, 8)
            er_stash(fcl, werC1, erC1s, CT, 16)

            # ---- batched projection pass (multi-job: shared lhs loads)
            def proj(lhs_ap_fn, n_tiles, jobs, BT=8):
                """lhs_ap_fn(k, c0, n) -> DRAM AP [P, n] for k-chunk cols.
                jobs: list of (wpair, tab)."""
                for t0 in range(0, n_tiles, BT):
                    bt = min(BT, n_tiles - t0)
                    lh = pl.tile([P, 2, bt * P], BF16, tag="projlh")
                    nc.sync.dma_start(lh[:, 0, :], lhs_ap_fn(0, t0 * P, bt * P))
                    nc.sync.dma_start(lh[:, 1, :], lhs_ap_fn(1, t0 * P, bt * P))
                    for j, (wpair, tab) in enumerate(jobs):
                        ob = pl.tile([P, bt, 264], BF16, tag=f"projout{j}")
                        for i in range(bt):
                            pp = ps.tile([P, 264], F32, tag="projps")
                            nc.tensor.matmul(pp[:],
                                             lhsT=lh[:, 0, i * P:(i + 1) * P],
                                             rhs=wpair[0][:],
                                             start=True, stop=False)
                            nc.tensor.matmul(pp[:],
                                             lhsT=lh[:, 1, i * P:(i + 1) * P],
                                             rhs=wpair[1][:],
                                             start=False, stop=True)
                            nc.scalar.copy(ob[:, i, :], pp[:])
                        nc.scalar.dma_start(
                            tab[t0 * P:(t0 + bt) * P, 0:264]
                            .rearrange("(t p) c -> p t c", p=P), ob[:])

            def dram_lhs(apx):
                return lambda k, c0, n: apx[k, :, c0:c0 + n]

            # L1 projections: tabDD+tabDC share feature loads
            proj(dram_lhs(featDT), NDT, [(wt['WDD'], tabDD.opt()),
                                         (wt['WDC'], tabDC.opt())])
            proj(dram_lhs(featCT), NCT, [(wt['WCC1'], tabCC1.opt())])

            # ---- edge phase
            def edge_phase(rel, tab_halves, er_fn, nchs, epilogue):
                si = SI[rel]
                S_dram, S2_dram = streams[rel][1], streams[rel][2]
                T = nchs.shape[0]
                col = 0
                for t in range(T):
                    segs, tnch = _tile_segments(nchs[t])
                    mps = psm.tile([P, 264], F32, tag="mainps")
                    er_t = er_fn(t)
                    ginfo = []
                    for g0 in range(0, tnch, GSZ):
                        gn = min(GSZ, tnch - g0)
                        G = gb.tile([P, gn, TW], BF16, tag="G")
                        for (s0, sn, h) in segs:
                            if s0 < g0 or s0 >= g0 + gn:
                                continue
                            nidx = sn * P
                            nc.gpsimd.dma_gather(
                                G[:, s0 - g0:s0 - g0 + sn, :], tab_halves[h],
                                si[:, (col + s0) * 8:(col + s0 + sn) * 8],
                                nidx, nidx, TW)
                        S = eb.tile([P, gn * P], FP8, tag="S")
                        nc.sync.dma_start(
                            S[:], S_dram[:, (col + g0) * P:(col + g0 + gn) * P])
                        S2 = eb.tile([P, gn * P], FP8, tag="S2")
                        nc.scalar.dma_start(
                            S2[:], S2_dram[:, (col + g0) * P:(col + g0 + gn) * P])
                        erps = ps.tile([P, gn * 8], F32, tag="erps")
                        for k in range(gn):
                            nc.tensor.matmul(
                                erps[:, k * 8:(k + 1) * 8],
                                lhsT=S2[:, k * P:(k + 1) * P], rhs=er_t,
                                start=True, stop=True, skip_group_check=True)
                        ww = eb.tile([P, gn, 8], F32, tag="ww")
                        nc.vector.tensor_tensor(
                            out=ww[:], in0=G[:, :, FD:FD + 8],
                            in1=erps[:].rearrange("p (a b) -> p a b", b=8),
                            op=AluOp.add)
                        nc.vector.scalar_tensor_tensor(
                            out=ww[:], in0=ww[:], scalar=0.2, in1=ww[:],
                            op0=AluOp.mult, op1=AluOp.max)
                        nc.scalar.activation(ww[:], ww[:], Act.Exp)
                        rhs = eb.tile([P, gn, 264], BF16, tag="rhs")
                        nc.vector.tensor_tensor(
                            out=rhs[:, :, 0:FD]
                                .rearrange("p a (h e) -> p a h e", h=H),
                            in0=G[:, :, 0:FD]
                                .rearrange("p a (h e) -> p a h e", h=H),
                            in1=ww[:, :, :, None].to_broadcast([P, gn, H, 32]),
                            op=AluOp.mult)
                        nc.vector.tensor_copy(rhs[:, :, FD:FD + 8], ww[:])
                        ginfo.append((g0, gn, S, rhs))
                    # main accumulation chain after all er/rhs prep of the
                    # tile: PE never head-of-line blocks the next group's
                    # er matmuls behind a stalled main.
                    for (g0, gn, S, rhs) in ginfo:
                        for k in range(gn):
                            nc.tensor.matmul(
                                mps[:], lhsT=S[:, k * P:(k + 1) * P],
                                rhs=rhs[:, k, :],
                                start=(g0 == 0 and k == 0),
                                stop=(g0 + gn == tnch and k == gn - 1),
                                skip_group_check=True)
                    epilogue(t, mps)
                    col += tnch

            def _norm_elu(pp, bias_tile, o_bf):
                den = sb.tile([P, 8], F32, tag="den")
                nc.vector.tensor_scalar_max(den[:], pp[:, FD:FD + 8], 1e-30)
                rec = sb.tile([P, 8], F32, tag="rec")
                nc.vector.reciprocal(rec[:], den[:])
                x = sb.tile([P, FD], F32, tag="xnrm")
                nc.vector.tensor_tensor(
                    out=x[:].rearrange("p (h e) -> p h e", h=H),
                    in0=pp[:, 0:FD].rearrange("p (h e) -> p h e", h=H),
                    in1=rec[:, :, None].to_broadcast([P, H, 32]),
                    op=AluOp.mult)
                nc.vector.tensor_add(x[:], x[:], bias_tile[:])
                ex = sb.tile([P, FD], F32, tag="eluex")
                nc.scalar.activation(ex[:], x[:], Act.Exp)
                nc.scalar.activation(ex[:], ex[:], Act.Relu, bias=1.0, scale=-1.0)
                xp = sb.tile([P, FD], F32, tag="elup")
                nc.scalar.activation(xp[:], x[:], Act.Relu)
                nc.vector.tensor_sub(o_bf[:], xp[:], ex[:])

            def _tstore(o_bf, dramT, t):
                ts = sb.tile([P, 2, P], BF16, tag="tpsb")
                for kk in range(2):
                    tp = ps.tile([P, P], BF16, space="PSUM", tag="aux")
                    nc.tensor.transpose(tp[:], o_bf[:, kk * P:(kk + 1) * P],
                                        C['ident'][:])
                    nc.scalar.copy(ts[:, kk, :], tp[:])
                nc.scalar.dma_start(
                    dramT[:, t * P:(t + 1) * P]
                    .rearrange("(a p) n -> p a n", p=P), ts[:])

            def make_epi(bias_idx, dramT):
                def epi(t, mps):
                    o = sb.tile([P, FD], BF16, tag="oed")
                    _norm_elu(mps, btiles[bias_idx], o)
                    _tstore(o, dramT, t)
                return epi

            # ---- L1 edge phases
            edge_phase('dd',
                       [tabDD.opt()[0:HALF, :], tabDD.opt()[HALF:NDP, :]],
                       lambda t: erDDs[:, t * 8:(t + 1) * 8],
                       sched['dd'], make_epi(0, hd1T.opt()))
            # local tabDC2 shard from local hd1, then AllGather the table
            proj(dram_lhs(hd1T.opt().rearrange("(a p) n -> a p n", p=P)),
                 DD_T, [(wt['WDC2'], tabDC2loc.opt())])
            nc.gpsimd.collective_compute(
                "AllGather", AluOp.bypass,
                replica_groups=[list(range(nco))],
                ins=[tabDC2loc.opt()], outs=[tabDC2.opt()])
            edge_phase('dc',
                       [tabDC.opt()[0:HALF, :], tabDC.opt()[HALF:NDP, :]],
                       lambda t: erC1s[:, t * 16:t * 16 + 8],
                       sched['dc'], make_epi(1, oDC1T.opt()))
            edge_phase('cc', [tabCC1.opt()[:, :]],
                       lambda t: erC1s[:, t * 16 + 8:t * 16 + 16],
                       sched['cc'], make_epi(2, oCC1T.opt()))

            # ---- semantic attention (cells) + optional er stash / head
            def sem_tile(l, oDCT, oCCT, t, consume):
                z = []
                for srcT in (oDCT, oCCT):
                    zt = sb.tile([P, 2, P], BF16, tag="semz")
                    nc.sync.dma_start(
                        zt[:], srcT[:, t * P:(t + 1) * P]
                        .rearrange("(a p) n -> p a n", p=P))
                    z.append(zt)
                wms = []
                for m in range(2):
                    hp = ps.tile([P, P], F32, tag="aux")
                    nc.tensor.matmul(hp[:], lhsT=sw1[l][0][:], rhs=z[m][:, 0, :],
                                     start=True, stop=False)
                    nc.tensor.matmul(hp[:], lhsT=sw1[l][1][:], rhs=z[m][:, 1, :],
                                     start=False, stop=True)
                    ht = sb.tile([P, P], BF16, tag="semh")
                    nc.scalar.activation(ht[:], hp[:], Act.Tanh, bias=sb1[l][:])
                    wp = ps.tile([1, P], F32, tag="aux")
                    nc.tensor.matmul(wp[:], lhsT=sw2[l][:], rhs=ht[:],
                                     start=True, stop=True)
                    wm = sb.tile([1, P], F32, tag="semw")
                    nc.scalar.copy(wm[:], wp[:])
                    wms.append(wm)
                beta = sb.tile([1, P], BF16, tag="semb")
                nc.vector.tensor_sub(beta[:], wms[0][:], wms[1][:])
                nc.scalar.activation(beta[:], beta[:], Act.Sigmoid)
                bb = ps.tile([P, P], F32, tag="aux")
                nc.tensor.matmul(bb[:], lhsT=ones1[:], rhs=beta[:],
                                 start=True, stop=True)
                hcts = []
                for kk in range(2):
                    diff = sb.tile([P, P], BF16, tag="semd")
                    nc.vector.tensor_sub(diff[:], z[0][:, kk, :], z[1][:, kk, :])
                    nc.vector.tensor_mul(diff[:], diff[:], bb[:])
                    hct = sb.tile([P, P], BF16, tag="semhc")
                    nc.vector.tensor_add(hct[:], z[1][:, kk, :], diff[:])
                    hcts.append(hct)
                consume(t, hcts)

            def sem1_consume(t, hcts):
                ht = sb.tile([P, 2, P], BF16, tag="hc1w")
                nc.vector.tensor_copy(ht[:, 0, :], hcts[0][:])
                nc.vector.tensor_copy(ht[:, 1, :], hcts[1][:])
                nc.sync.dma_start(
                    hc1T.opt()[:, t * P:(t + 1) * P]
                    .rearrange("(a p) n -> p a n", p=P), ht[:])
                ep = ps.tile([P, 16], F32, tag="aux")
                nc.tensor.matmul(ep[:], lhsT=hcts[0][:], rhs=werC2[0][:],
                                 start=True, stop=False)
                nc.tensor.matmul(ep[:], lhsT=hcts[1][:], rhs=werC2[1][:],
                                 start=False, stop=True)
                nc.scalar.copy(erC2s[:, t * 16:(t + 1) * 16], ep[:])

            for t in range(CT):
                sem_tile(0, oDC1T.opt(), oCC1T.opt(), t, sem1_consume)
            # local tabCC2 shard from local hc1, then AllGather the table
            proj(dram_lhs(hc1T.opt().rearrange("(a p) n -> a p n", p=P)),
                 CT, [(wt['WCC2'], tabCC2loc.opt())], BT=CT)
            nc.gpsimd.collective_compute(
                "AllGather", AluOp.bypass,
                replica_groups=[list(range(nco))],
                ins=[tabCC2loc.opt()], outs=[tabCC2.opt()])

            # ---- L2 edge phases
            edge_phase('dc',
                       [tabDC2.opt()[0:HALF, :], tabDC2.opt()[HALF:NDP, :]],
                       lambda t: erC2s[:, t * 16:t * 16 + 8],
                       sched['dc'], make_epi(3, oDC2T.opt()))
            edge_phase('cc', [tabCC2.opt()[:, :]],
                       lambda t: erC2s[:, t * 16 + 8:t * 16 + 16],
                       sched['cc'], make_epi(4, oCC2T.opt()))

            # ---- sem2 + MLP head
            def sem2_consume(t, hcts):
                h1p = ps.tile([32, P], F32, tag="aux")
                nc.tensor.matmul(h1p[:], lhsT=dW1[0][:], rhs=hcts[0][:],
                                 start=True, stop=False)
                nc.tensor.matmul(h1p[:], lhsT=dW1[1][:], rhs=hcts[1][:],
                                 start=False, stop=True)
                h1 = sb.tile([32, P], BF16, tag="mlph1")
                nc.vector.scalar_tensor_tensor(
                    out=h1[:], in0=h1p[:], scalar=1.0,
                    in1=db1[:].to_broadcast([32, P]),
                    op0=AluOp.mult, op1=AluOp.add)
                nc.vector.scalar_tensor_tensor(
                    out=h1[:], in0=h1[:], scalar=0.01, in1=h1[:],
                    op0=AluOp.mult, op1=AluOp.max)
                h2p = ps.tile([16, P], F32, tag="aux")
                nc.tensor.matmul(h2p[:], lhsT=dW2[:], rhs=h1[:],
                                 start=True, stop=True)
                h2 = sb.tile([16, P], BF16, tag="mlph2")
                nc.vector.scalar_tensor_tensor(
                    out=h2[:], in0=h2p[:], scalar=1.0,
                    in1=db2[:].to_broadcast([16, P]),
                    op0=AluOp.mult, op1=AluOp.add)
                nc.vector.scalar_tensor_tensor(
                    out=h2[:], in0=h2[:], scalar=0.01, in1=h2[:],
                    op0=AluOp.mult, op1=AluOp.max)
                h3p = ps.tile([1, P], F32, tag="aux")
                nc.tensor.matmul(h3p[:], lhsT=dW3[:], rhs=h2[:],
                                 start=True, stop=True)
                h3 = sb.tile([1, P], F32, tag="mlph3")
                nc.vector.tensor_scalar(h3[:], h3p[:], db3[:], None,
                                        op0=AluOp.add)
                nc.sync.dma_start(out[0:1, t * P:(t + 1) * P], h3[:])

            for t in range(CT):
                sem_tile(1, oDC2T.opt(), oCC2T.opt(), t, sem2_consume)

    nc.compile()
    if legalize:
        legalize_waits(nc)
    return nc


# --------------------------------------------------------------------------
# entry point
# --------------------------------------------------------------------------

_CACHE = {}


def kernel(**inputs):
    cfg = make_cfg(inputs['feat_drug'].shape[0], inputs['feat_cell'].shape[0])
    sched, in_maps = host_prep(inputs, cfg)
    key = tuple(int(x) for s in sched.values() for x in s.flatten())
    if key not in _CACHE:
        _CACHE[key] = build_program(sched, cfg)
    nc = _CACHE[key]
    from concourse.bass_utils import run_bass_kernel_spmd
    res = run_bass_kernel_spmd(nc, in_maps, list(range(cfg['n_cores'])))
    pieces = [res.results[c]['out'][0] for c in range(cfg['n_cores'])]
    full = np.concatenate([p[:cfg['CBLK']] for p in pieces])[:cfg['Nc']]
    return full.reshape(-1, 1).astype(np.float32)


# revision 26
# speedup vs baseline: 1.0210x; 1.0210x over previous
"""Trainium2 Bass kernel v2 for the 2-layer heterogeneous GAT (drug/cell).

Strategy (8 NeuronCores, SPMD single program), changes vs v1:
  - All five edge phases partitioned by DST block; dc2 now dst-partitioned
    too, fed by an AllGather of hd1 (bf16) instead of AllToAll partials.
  - bf16 gather tables, 384-col rows (768B, %256B) holding [fs 256 | el 8].
  - er is never gathered: per-dst-tile er vectors [128, 8] are stashed in
    SBUF (projected from local features / hc1 tiles) and broadcast to edges
    with a one-hot matmul (lhsT = S2[dst -> edge]).
  - Edge phase batches DVE work per group of <=16 chunks: one is_equal for
    all S chunks, one for S2 (dst-row stream replicated via DMA broadcast),
    one fused exp/leaky pipeline, one rhs build.
  - Gathers up to 1024 idx per call (SWDGE ring limit is < 2048).
  - Projections batched: 8 tiles per DMA load/store, psum->SBUF copies
    alternate ACT/DVE.
"""
import sys
sys.path.insert(0, '/opt/trn_rl_repo')
import numpy as np
import ml_dtypes

import concourse.bacc as bacc
import concourse.tile as tile
from concourse import mybir, library_config

F32 = mybir.dt.float32
BF16 = mybir.dt.bfloat16
FP8 = mybir.dt.float8e4
I16 = mybir.dt.int16
BF = ml_dtypes.bfloat16
F8 = mybir.dt.np(FP8)
P = 128
H = 8
FD = 256          # feature dim
TW = 384          # gather-table row width (bf16) = 768B; payload 264
GSZ = 16          # chunks per batched edge group
GCH = 8           # chunks per dma_gather call (1024 idx; 2048 hangs)
AluOp = mybir.AluOpType
Act = mybir.ActivationFunctionType

N_CORES = 8


def legalize_waits(nc):
    """Split multi-wait instructions into wait-carrying NOP chains."""
    n_split = 0
    for fn in nc.m.functions:
        for bb in fn.blocks:
            insts = bb.instructions
            new = []
            changed = False
            for inst in insts:
                si = inst.sync_info
                waits = list(si.on_wait) if si is not None else []
                cap = 2 if isinstance(inst, mybir.InstEventSemaphore) else 1
                if len(waits) > cap:
                    keep = waits[-cap:]
                    for w in waits[:-cap]:
                        nop = mybir.InstNoOp(
                            name=nc.get_next_instruction_name(),
                            engine=inst.engine,
                            sync_info=mybir.SyncInfo(on_wait=[w], on_update=[]),
                            bass_nofuse=True,
                        )
                        new.append(nop)
                        n_split += 1
                    inst.sync_info = mybir.SyncInfo(
                        on_wait=keep, on_update=list(si.on_update))
                    changed = True
                new.append(inst)
            if changed:
                bb.instructions = new
    return n_split


def make_cfg(Nd, Nc):
    ndp = -(-Nd // 1024) * 1024
    ncp = -(-Nc // 1024) * 1024
    return dict(Nd=Nd, Nc=Nc, NDP=ndp, NCP=ncp, DBLK=ndp // 8,
                CBLK=ncp // 8, HALF=ndp // 2, n_cores=8)


# --------------------------------------------------------------------------
# host-side prep
# --------------------------------------------------------------------------

def _fold_weights(ip):
    def wel(W, a):  # W [256,256], a [H,D] -> [256,H]
        return (np.asarray(W, np.float32).reshape(FD, H, -1)
                * np.asarray(a, np.float32)[None]).sum(-1)
    Wsrc = np.asarray(ip['Wsrc'], np.float32)
    Wdst = np.asarray(ip['Wdst_dc'], np.float32)
    al = np.asarray(ip['attn_l'], np.float32)
    ar = np.asarray(ip['attn_r'], np.float32)

    def w264(W, a):
        return np.concatenate([W, wel(W, a)], 1)  # [256, 264]

    def bftile(a, ncols):  # [256, ncols] -> [2, 128, ncols] bf16
        return np.ascontiguousarray(a.reshape(2, P, ncols)).astype(BF)

    out = {}
    out['WDD'] = bftile(w264(Wsrc[0, 0], al[0, 0]), 264)
    out['WDC'] = bftile(w264(Wsrc[0, 1], al[0, 1]), 264)
    out['WCC1'] = bftile(w264(Wsrc[0, 2], al[0, 2]), 264)
    out['WDC2'] = bftile(w264(Wsrc[1, 1], al[1, 1]), 264)
    out['WCC2'] = bftile(w264(Wsrc[1, 2], al[1, 2]), 264)
    out['WerDD'] = bftile(wel(Wsrc[0, 0], ar[0, 0]), 8)
    out['WerC1'] = bftile(np.concatenate(
        [wel(Wdst[0], ar[0, 1]), wel(Wsrc[0, 2], ar[0, 2])], 1), 16)
    out['WerC2'] = bftile(np.concatenate(
        [wel(Wdst[1], ar[1, 1]), wel(Wsrc[1, 2], ar[1, 2])], 1), 16)
    out['bias5'] = np.stack([ip['gat_bias'][0, 0], ip['gat_bias'][0, 1],
                             ip['gat_bias'][0, 2], ip['gat_bias'][1, 1],
                             ip['gat_bias'][1, 2]]).astype(np.float32)
    out['semW1'] = np.stack([np.asarray(ip['sem_W1'][l, 1], np.float32)
                             .reshape(2, P, P) for l in range(2)]).astype(BF)
    out['semb1'] = np.stack([np.asarray(ip['sem_b1'][l, 1], np.float32)
                             .reshape(P, 1) for l in range(2)]).astype(np.float32)
    out['semW2'] = np.stack([np.asarray(ip['sem_W2'][l, 1], np.float32)
                             .reshape(P, 1) for l in range(2)]).astype(BF)
    out['dnnW1'] = np.asarray(ip['dnn_W1'], np.float32).reshape(2, P, 32).astype(BF)
    out['dnnb1'] = np.asarray(ip['dnn_b1'], np.float32).reshape(32, 1)
    out['dnnW2'] = np.asarray(ip['dnn_W2'], np.float32).astype(BF)
    out['dnnb2'] = np.asarray(ip['dnn_b2'], np.float32).reshape(16, 1)
    out['dnnW3'] = np.asarray(ip['dnn_W3'], np.float32).astype(BF)
    out['dnnb3'] = np.asarray(ip['dnn_b3'], np.float32).reshape(1, 1)
    return out


def _wrap16(vals):
    n = len(vals)
    out = np.zeros((P, n // 16), np.int16)
    a = np.asarray(vals, np.int16).reshape(-1, 16).T
    for g in range(8):
        out[g * 16:(g + 1) * 16, :] = a
    return out


def _prep_rel(src, dst, nco, *, blk, n_tiles, halves):
    """Per-core edge schedule, dst-partitioned.

    Returns nch [T, n_halves] (equalized over cores) and per-core streams:
    src idx wrapped i16 [P, tot*8], plus host-built one-hot scatter
    matrices in fp8: S [P, tot*128] (S[p, k*128+d] = dl[p,k]==d) and
    S2 [P, tot*128] (S2[p, k*128+e] = dl[e,k]==p).
    """
    src = np.asarray(src, np.int64)
    dst = np.asarray(dst, np.int64)
    per_core = []
    for c in range(nco):
        m = (dst >= c * blk) & (dst < (c + 1) * blk)
        dl = dst[m] - c * blk
        s = src[m]
        tid = dl // P
        per_core.append((s, dl % P, tid))
    nh = 2 if halves else 1
    nch = np.zeros((n_tiles, nh), np.int64)
    buckets = []
    for c in range(nco):
        s, dloc, tid = per_core[c]
        bk = {}
        for t in range(n_tiles):
            mt = tid == t
            st, dt_ = s[mt], dloc[mt]
            if halves:
                m0 = st < halves
                groups = [(st[m0], dt_[m0]), (st[~m0] - halves, dt_[~m0])]
            else:
                groups = [(st, dt_)]
            bk[t] = groups
            for h, (gs, gd) in enumerate(groups):
                nch[t, h] = max(nch[t, h], (len(gs) + P - 1) // P)
        buckets.append(bk)
    nch = np.maximum(nch, 1)
    tot = int(nch.sum())
    rng = np.arange(P, dtype=np.int64)
    srcs, Ss, S2s = [], [], []
    for c in range(nco):
        bk = buckets[c]
        sw = np.zeros((P, tot * 8), np.int16)
        dall = np.full((tot, P), -1, np.int64)   # [chunk, edge] local dst
        col = 0
        for t in range(n_tiles):
            for h, (gs, gd) in enumerate(bk[t]):
                n = int(nch[t, h]) * P
                a = np.zeros(n, np.int64)
                a[:len(gs)] = gs
                assert a.max(initial=0) < 32768
                sw[:, col * 8:(col + int(nch[t, h])) * 8] = _wrap16(a)
                dpad = np.full(n, -1, np.int64)
                dpad[:len(gd)] = gd
                dall[col:col + int(nch[t, h])] = dpad.reshape(-1, P)
                col += int(nch[t, h])
        # S[p, k, d] = (dall[k, p] == d);  S2[p, k, e] = (dall[k, e] == p)
        S = (dall.T[:, :, None] == rng[None, None, :]).astype(F8)
        S2 = (dall[None, :, :] == rng[:, None, None]).astype(F8)
        srcs.append(sw)
        Ss.append(np.ascontiguousarray(S.reshape(P, tot * P)))
        S2s.append(np.ascontiguousarray(S2.reshape(P, tot * P)))
    return dict(nch=nch, src=srcs, S=Ss, S2=S2s, tot=tot)


def host_prep(ip, cfg):
    W = _fold_weights(ip)
    nco = cfg['n_cores']
    DBLK, CBLK, HALF = cfg['DBLK'], cfg['CBLK'], cfg['HALF']
    NDP, NCP = cfg['NDP'], cfg['NCP']
    DD_T, CT = DBLK // P, CBLK // P

    dd = _prep_rel(ip['src_dd'], ip['dst_dd'], nco, blk=DBLK,
                   n_tiles=DD_T, halves=HALF)
    dc = _prep_rel(ip['src_dc'], ip['dst_dc'], nco, blk=CBLK,
                   n_tiles=CT, halves=HALF)
    cc = _prep_rel(ip['src_cc'], ip['dst_cc'], nco, blk=CBLK,
                   n_tiles=CT, halves=None)

    featD = np.zeros((NDP, FD), np.float32)
    featD[:cfg['Nd']] = np.asarray(ip['feat_drug'], np.float32)
    featC = np.zeros((NCP, FD), np.float32)
    featC[:cfg['Nc']] = np.asarray(ip['feat_cell'], np.float32)
    featDT = np.ascontiguousarray(featD.T).reshape(2, P, NDP).astype(BF)
    featCT = np.ascontiguousarray(featC.T).reshape(2, P, NCP).astype(BF)

    identB = np.eye(P, dtype=np.float32).astype(BF)

    base = dict(featDT=featDT, featCT=featCT, identB=identB, **W)
    in_maps = []
    for c in range(nco):
        m = dict(base)
        m['featDTloc'] = np.ascontiguousarray(
            featD[c * DBLK:(c + 1) * DBLK].T).reshape(2, P, DBLK).astype(BF)
        m['featCTloc'] = np.ascontiguousarray(
            featC[c * CBLK:(c + 1) * CBLK].T).reshape(2, P, CBLK).astype(BF)
        for r, d in (('dd', dd), ('dc', dc), ('cc', cc)):
            m[f'{r}_src'] = d['src'][c]
            m[f'{r}_S'] = d['S'][c]
            m[f'{r}_S2'] = d['S2'][c]
        in_maps.append(m)
    sched = dict(dd=dd['nch'], dc=dc['nch'], cc=cc['nch'])
    return sched, in_maps


# --------------------------------------------------------------------------
# device program
# --------------------------------------------------------------------------

def _tile_segments(nch_row):
    """Per-tile gather segments [(start_col, n, half)], respecting half
    boundaries, GSZ group windows, and GCH call caps."""
    bounds = []
    off = 0
    for h, nh in enumerate(nch_row):
        bounds.append((off, off + int(nh), h))
        off += int(nh)
    tnch = off
    segs = []
    for g0 in range(0, tnch, GSZ):
        g1 = min(g0 + GSZ, tnch)
        for (h0, h1, h) in bounds:
            s0 = max(g0, h0)
            s1 = min(g1, h1)
            for b in range(s0, s1, GCH):
                segs.append((b, min(GCH, s1 - b), h))
    return segs, tnch


def build_program(sched, cfg, legalize=True):
    nco = cfg['n_cores']
    DBLK, CBLK, HALF = cfg['DBLK'], cfg['CBLK'], cfg['HALF']
    NDP, NCP = cfg['NDP'], cfg['NCP']
    DD_T, CT = DBLK // P, CBLK // P
    NDT, NCT = NDP // P, NCP // P

    nc = bacc.Bacc(None)
    d = {}
    def inp(name, shape, dt=BF16):
        d[name] = nc.declare_dram_parameter(name, list(shape), dt,
                                            isOutput=False)
        return d[name]

    featDT = inp('featDT', (2, P, NDP))
    featCT = inp('featCT', (2, P, NCP))
    featDTloc = inp('featDTloc', (2, P, DBLK))
    featCTloc = inp('featCTloc', (2, P, CBLK))
    identB_in = inp('identB', (P, P))
    Wmain = {k: inp(k, (2, P, 264)) for k in
             ('WDD', 'WDC', 'WCC1', 'WDC2', 'WCC2')}
    WerDD = inp('WerDD', (2, P, 8))
    WerC1 = inp('WerC1', (2, P, 16))
    WerC2 = inp('WerC2', (2, P, 16))
    bias5 = inp('bias5', (5, FD), F32)
    semW1 = inp('semW1', (2, 2, P, P))
    semb1 = inp('semb1', (2, P, 1), F32)
    semW2 = inp('semW2', (2, P, 1))
    dnnW1 = inp('dnnW1', (2, P, 32)); dnnb1 = inp('dnnb1', (32, 1), F32)
    dnnW2 = inp('dnnW2', (32, 16)); dnnb2 = inp('dnnb2', (16, 1), F32)
    dnnW3 = inp('dnnW3', (16, 1)); dnnb3 = inp('dnnb3', (1, 1), F32)
    streams = {}
    for r, nchs in sched.items():
        tot = int(nchs.sum())
        streams[r] = (inp(f'{r}_src', (P, tot * 8), I16),
                      inp(f'{r}_S', (P, tot * P), FP8),
                      inp(f'{r}_S2', (P, tot * P), FP8))
    out = nc.declare_dram_parameter('out', [1, CBLK], F32, isOutput=True)

    with tile.TileContext(nc) as tc:
        with tc.tile_pool(name="const", bufs=1) as cpool, \
             tc.tile_pool(name="sb", bufs=3) as sb, \
             tc.tile_pool(name="pl", bufs=3) as pl, \
             tc.tile_pool(name="gb", bufs=3) as gb, \
             tc.tile_pool(name="eb", bufs=3) as eb, \
             tc.tile_pool(name="ps", bufs=2, space="PSUM") as ps, \
             tc.tile_pool(name="psm", bufs=2, space="PSUM") as psm, \
             tc.tile_pool(name="dram", bufs=1, space="DRAM") as dr:
            nc.gpsimd.load_library(library_config.mlp)

            # ---- constants in SBUF
            C = {}
            def cload(name, ap, shape, dt=BF16):
                t_ = cpool.tile(list(shape), dt, tag=f"c_{name}")
                nc.sync.dma_start(t_[:], ap)
                return t_
            C['ident'] = cload('ident', identB_in[:], (P, P))
            ones1 = cpool.tile([1, P], BF16)
            nc.vector.memset(ones1[:], 1.0)
            wt = {}
            for k in Wmain:
                wt[k] = tuple(cload(f'{k}{j}', Wmain[k][j], (P, 264))
                              for j in range(2))
            werDD = tuple(cload(f'werDD{j}', WerDD[j], (P, 8))
                          for j in range(2))
            werC1 = tuple(cload(f'werC1{j}', WerC1[j], (P, 16))
                          for j in range(2))
            werC2 = tuple(cload(f'werC2{j}', WerC2[j], (P, 16))
                          for j in range(2))
            btiles = [cload(f'bias{r}', bias5[r:r + 1, :].to_broadcast([P, FD]),
                            (P, FD), F32) for r in range(5)]
            sw1 = [tuple(cload(f'sw1_{l}{j}', semW1[l, j], (P, P))
                         for j in range(2)) for l in range(2)]
            sb1 = [cload(f'sb1_{l}', semb1[l], (P, 1), F32) for l in range(2)]
            sw2 = [cload(f'sw2_{l}', semW2[l], (P, 1)) for l in range(2)]
            dW1 = tuple(cload(f'dW1{j}', dnnW1[j], (P, 32)) for j in range(2))
            dW2 = cload('dW2', dnnW2[:], (32, 16))
            dW3 = cload('dW3', dnnW3[:], (16, 1))
            db1 = cload('db1', dnnb1[:], (32, 1), F32)
            db2 = cload('db2', dnnb2[:], (16, 1), F32)
            db3 = cload('db3', dnnb3[:], (1, 1), F32)
            # resident local features (for er projections)
            fdl = tuple(cload(f'fdl{j}', featDTloc[j], (P, DBLK))
                        for j in range(2))
            fcl = tuple(cload(f'fcl{j}', featCTloc[j], (P, CBLK))
                        for j in range(2))
            # resident edge streams
            SI = {}
            for r in sched:
                tot = int(sched[r].sum())
                SI[r] = cload(f'si_{r}', streams[r][0][:], (P, tot * 8), I16)
            # er stashes
            erDDs = cpool.tile([P, DD_T * 8], BF16, tag="erDDs")
            erC1s = cpool.tile([P, CT * 16], BF16, tag="erC1s")
            erC2s = cpool.tile([P, CT * 16], BF16, tag="erC2s")
            # transposed per-relation output stashes [P, tile, kchunk, node]
            oDC1s = cpool.tile([P, CT, 2, P], BF16, tag="oDC1s")
            oCC1s = cpool.tile([P, CT, 2, P], BF16, tag="oCC1s")
            oDC2s = cpool.tile([P, CT, 2, P], BF16, tag="oDC2s")
            oCC2s = cpool.tile([P, CT, 2, P], BF16, tag="oCC2s")

            # ---- internal DRAM
            tabDD = dr.tile([NDP, TW], BF16)
            tabDC = dr.tile([NDP, TW], BF16)
            tabCC1 = dr.tile([NCP, TW], BF16)
            tabDC2loc = dr.tile([DBLK, TW], BF16)
            tabDC2 = dr.tile([NDP, TW], BF16, addr_space="Shared")
            tabCC2loc = dr.tile([CBLK, TW], BF16)
            tabCC2 = dr.tile([NCP, TW], BF16, addr_space="Shared")

            # ---- er stash projections (local features)
            def er_stash(fres, wpair, stash, n_tiles, ncols):
                BT = min(8, n_tiles) if ncols == 8 else n_tiles
                for t0 in range(0, n_tiles, BT):
                    bt = min(BT, n_tiles - t0)
                    pp = ps.tile([P, bt * ncols], F32, tag="projps")
                    for i in range(bt):
                        tl = (t0 + i) * P
                        nc.tensor.matmul(pp[:, i * ncols:(i + 1) * ncols],
                                         lhsT=fres[0][:, tl:tl + P],
                                         rhs=wpair[0][:], start=True, stop=False)
                        nc.tensor.matmul(pp[:, i * ncols:(i + 1) * ncols],
                                         lhsT=fres[1][:, tl:tl + P],
                                         rhs=wpair[1][:], start=False, stop=True)
                    nc.scalar.copy(stash[:, t0 * ncols:(t0 + bt) * ncols], pp[:])
            er_stash(fdl, werDD, erDDs, DD_T, 8)
            er_stash(fcl, werC1, erC1s, CT, 16)

            # ---- batched projection pass (multi-job: shared lhs loads)
            def proj(lhs_ap_fn, n_tiles, jobs, BT=8):
                """lhs_ap_fn(k, c0, n) -> DRAM AP [P, n] for k-chunk cols.
                jobs: list of (wpair, tab)."""
                for t0 in range(0, n_tiles, BT):
                    bt = min(BT, n_tiles - t0)
                    lh = pl.tile([P, 2, bt * P], BF16, tag="projlh")
                    nc.sync.dma_start(lh[:, 0, :], lhs_ap_fn(0, t0 * P, bt * P))
                    nc.sync.dma_start(lh[:, 1, :], lhs_ap_fn(1, t0 * P, bt * P))
                    for j, (wpair, tab) in enumerate(jobs):
                        ob = pl.tile([P, bt, 264], BF16, tag=f"projout{j}")
                        for i in range(bt):
                            pp = ps.tile([P, 264], F32, tag="projps")
                            nc.tensor.matmul(pp[:],
                                             lhsT=lh[:, 0, i * P:(i + 1) * P],
                                             rhs=wpair[0][:],
                                             start=True, stop=False)
                            nc.tensor.matmul(pp[:],
                                             lhsT=lh[:, 1, i * P:(i + 1) * P],
                                             rhs=wpair[1][:],
                                             start=False, stop=True)
                            nc.scalar.copy(ob[:, i, :], pp[:])
                        nc.sync.dma_start(
                            tab[t0 * P:(t0 + bt) * P, 0:264]
                            .rearrange("(t p) c -> p t c", p=P), ob[:])

            def dram_lhs(apx):
                return lambda k, c0, n: apx[k, :, c0:c0 + n]

            # L1 projections: tabDD+tabDC share feature loads
            proj(dram_lhs(featDT), NDT, [(wt['WDD'], tabDD.opt()),
                                         (wt['WDC'], tabDC.opt())])
            proj(dram_lhs(featCT), NCT, [(wt['WCC1'], tabCC1.opt())])

            # ---- edge phase
            def edge_phase(rel, tab_halves, er_fn, nchs, epilogue):
                si = SI[rel]
                S_dram, S2_dram = streams[rel][1], streams[rel][2]
                T = nchs.shape[0]
                col = 0
                for t in range(T):
                    segs, tnch = _tile_segments(nchs[t])
                    mps = psm.tile([P, 264], F32, tag="mainps")
                    er_t = er_fn(t)
                    ginfo = []
                    for g0 in range(0, tnch, GSZ):
                        gn = min(GSZ, tnch - g0)
                        G = gb.tile([P, gn, TW], BF16, tag="G")
                        for (s0, sn, h) in segs:
                            if s0 < g0 or s0 >= g0 + gn:
                                continue
                            nidx = sn * P
                            nc.gpsimd.dma_gather(
                                G[:, s0 - g0:s0 - g0 + sn, :], tab_halves[h],
                                si[:, (col + s0) * 8:(col + s0 + sn) * 8],
                                nidx, nidx, TW)
                        S = eb.tile([P, gn * P], FP8, tag="S")
                        nc.sync.dma_start(
                            S[:], S_dram[:, (col + g0) * P:(col + g0 + gn) * P])
                        S2 = eb.tile([P, gn * P], FP8, tag="S2")
                        nc.sync.dma_start(
                            S2[:], S2_dram[:, (col + g0) * P:(col + g0 + gn) * P])
                        erps = ps.tile([P, gn * 8], F32, tag="erps")
                        for k in range(gn):
                            nc.tensor.matmul(
                                erps[:, k * 8:(k + 1) * 8],
                                lhsT=S2[:, k * P:(k + 1) * P], rhs=er_t,
                                start=True, stop=True, skip_group_check=True)
                        ww = eb.tile([P, gn, 8], F32, tag="ww")
                        nc.vector.tensor_tensor(
                            out=ww[:], in0=G[:, :, FD:FD + 8],
                            in1=erps[:].rearrange("p (a b) -> p a b", b=8),
                            op=AluOp.add)
                        nc.vector.scalar_tensor_tensor(
                            out=ww[:], in0=ww[:], scalar=0.2, in1=ww[:],
                            op0=AluOp.mult, op1=AluOp.max)
                        nc.scalar.activation(ww[:], ww[:], Act.Exp)
                        rhs = eb.tile([P, gn, 264], BF16, tag="rhs")
                        nc.vector.tensor_tensor(
                            out=rhs[:, :, 0:FD]
                                .rearrange("p a (h e) -> p a h e", h=H),
                            in0=G[:, :, 0:FD]
                                .rearrange("p a (h e) -> p a h e", h=H),
                            in1=ww[:, :, :, None].to_broadcast([P, gn, H, 32]),
                            op=AluOp.mult)
                        nc.vector.tensor_copy(rhs[:, :, FD:FD + 8], ww[:])
                        ginfo.append((g0, gn, S, rhs))
                    # main accumulation chain after all er/rhs prep of the
                    # tile: PE never head-of-line blocks the next group's
                    # er matmuls behind a stalled main.
                    for (g0, gn, S, rhs) in ginfo:
                        for k in range(gn):
                            nc.tensor.matmul(
                                mps[:], lhsT=S[:, k * P:(k + 1) * P],
                                rhs=rhs[:, k, :],
                                start=(g0 == 0 and k == 0),
                                stop=(g0 + gn == tnch and k == gn - 1),
                                skip_group_check=True)
                    epilogue(t, mps)
                    col += tnch

            def _norm_elu(pp, bias_tile, o_bf):
                den = sb.tile([P, 8], F32, tag="den")
                nc.vector.tensor_scalar_max(den[:], pp[:, FD:FD + 8], 1e-30)
                rec = sb.tile([P, 8], F32, tag="rec")
                nc.vector.reciprocal(rec[:], den[:])
                x = sb.tile([P, FD], F32, tag="xnrm")
                nc.vector.tensor_tensor(
                    out=x[:].rearrange("p (h e) -> p h e", h=H),
                    in0=pp[:, 0:FD].rearrange("p (h e) -> p h e", h=H),
                    in1=rec[:, :, None].to_broadcast([P, H, 32]),
                    op=AluOp.mult)
                nc.vector.tensor_add(x[:], x[:], bias_tile[:])
                ex = sb.tile([P, FD], F32, tag="eluex")
                nc.scalar.activation(ex[:], x[:], Act.Exp)
                nc.scalar.activation(ex[:], ex[:], Act.Relu, bias=1.0, scale=-1.0)
                xp = sb.tile([P, FD], F32, tag="elup")
                nc.scalar.activation(xp[:], x[:], Act.Relu)
                nc.vector.tensor_sub(o_bf[:], xp[:], ex[:])

            def _transp(o_bf, kk):
                tp = ps.tile([P, P], BF16, space="PSUM", tag="aux")
                nc.tensor.transpose(tp[:], o_bf[:, kk * P:(kk + 1) * P],
                                    C['ident'][:])
                return tp

            def make_epi_stash(bias_idx, stash):
                def epi(t, mps):
                    o = sb.tile([P, FD], BF16, tag="oed")
                    _norm_elu(mps, btiles[bias_idx], o)
                    for kk in range(2):
                        tp = _transp(o, kk)
                        nc.scalar.copy(stash[:, t, kk, :], tp[:])
                return epi

            # dd epilogue fuses the tabDC2 local-shard projection: the
            # transposed hd1 tile is exactly the proj lhsT, so hd1 never
            # round-trips through DRAM.
            ddep_state = {}
            def dd_epi(t, mps, T=DD_T):
                rs = min(8, T)
                o = sb.tile([P, FD], BF16, tag="oed")
                _norm_elu(mps, btiles[0], o)
                ts = sb.tile([P, 2, P], BF16, tag="tpsb")
                for kk in range(2):
                    tp = _transp(o, kk)
                    nc.scalar.copy(ts[:, kk, :], tp[:])
                pp2 = ps.tile([P, 264], F32, tag="projps")
                nc.tensor.matmul(pp2[:], lhsT=ts[:, 0, :], rhs=wt['WDC2'][0][:],
                                 start=True, stop=False)
                nc.tensor.matmul(pp2[:], lhsT=ts[:, 1, :], rhs=wt['WDC2'][1][:],
                                 start=False, stop=True)
                if t % rs == 0:
                    ob_dd = pl.tile([P, rs, 264], BF16, tag="projout0")
                    ddep_state['ob'] = ob_dd
                nc.scalar.copy(ddep_state['ob'][:, t % rs, :], pp2[:])
                if t % rs == rs - 1 or t == T - 1:
                    cnt = t % rs + 1
                    t0 = t - t % rs
                    nc.sync.dma_start(
                        tabDC2loc.opt()[t0 * P:(t + 1) * P, 0:264]
                        .rearrange("(a p) c -> p a c", p=P),
                        ddep_state['ob'][:, 0:cnt, :])

            # ---- L1 edge phases
            edge_phase('dd',
                       [tabDD.opt()[0:HALF, :], tabDD.opt()[HALF:NDP, :]],
                       lambda t: erDDs[:, t * 8:(t + 1) * 8],
                       sched['dd'], dd_epi)
            nc.gpsimd.collective_compute(
                "AllGather", AluOp.bypass,
                replica_groups=[list(range(nco))],
                ins=[tabDC2loc.opt()], outs=[tabDC2.opt()])
            edge_phase('dc',
                       [tabDC.opt()[0:HALF, :], tabDC.opt()[HALF:NDP, :]],
                       lambda t: erC1s[:, t * 16:t * 16 + 8],
                       sched['dc'], make_epi_stash(1, oDC1s))
            edge_phase('cc', [tabCC1.opt()[:, :]],
                       lambda t: erC1s[:, t * 16 + 8:t * 16 + 16],
                       sched['cc'], make_epi_stash(2, oCC1s))

            # ---- semantic attention (cells) + optional er stash / head
            def sem_tile(l, oDCs_, oCCs_, t, consume):
                z = [oDCs_[:, t, :, :], oCCs_[:, t, :, :]]
                wms = []
                for m in range(2):
                    hp = ps.tile([P, P], F32, tag="aux")
                    nc.tensor.matmul(hp[:], lhsT=sw1[l][0][:], rhs=z[m][:, 0, :],
                                     start=True, stop=False)
                    nc.tensor.matmul(hp[:], lhsT=sw1[l][1][:], rhs=z[m][:, 1, :],
                                     start=False, stop=True)
                    ht = sb.tile([P, P], BF16, tag="semh")
                    nc.scalar.activation(ht[:], hp[:], Act.Tanh, bias=sb1[l][:])
                    wp = ps.tile([1, P], F32, tag="aux")
                    nc.tensor.matmul(wp[:], lhsT=sw2[l][:], rhs=ht[:],
                                     start=True, stop=True)
                    wm = sb.tile([1, P], F32, tag="semw")
                    nc.scalar.copy(wm[:], wp[:])
                    wms.append(wm)
                beta = sb.tile([1, P], BF16, tag="semb")
                nc.vector.tensor_sub(beta[:], wms[0][:], wms[1][:])
                nc.scalar.activation(beta[:], beta[:], Act.Sigmoid)
                bb = ps.tile([P, P], F32, tag="aux")
                nc.tensor.matmul(bb[:], lhsT=ones1[:], rhs=beta[:],
                                 start=True, stop=True)
                hcts = []
                for kk in range(2):
                    diff = sb.tile([P, P], BF16, tag="semd")
                    nc.vector.tensor_sub(diff[:], z[0][:, kk, :], z[1][:, kk, :])
                    nc.vector.tensor_mul(diff[:], diff[:], bb[:])
                    hct = sb.tile([P, P], BF16, tag="semhc")
                    nc.vector.tensor_add(hct[:], z[1][:, kk, :], diff[:])
                    hcts.append(hct)
                consume(t, hcts)

            # sem1 fuses the tabCC2 local-shard projection (hct is the lhsT)
            sem_state = {}
            def sem1_consume(t, hcts):
                ep = ps.tile([P, 16], F32, tag="aux")
                nc.tensor.matmul(ep[:], lhsT=hcts[0][:], rhs=werC2[0][:],
                                 start=True, stop=False)
                nc.tensor.matmul(ep[:], lhsT=hcts[1][:], rhs=werC2[1][:],
                                 start=False, stop=True)
                nc.scalar.copy(erC2s[:, t * 16:(t + 1) * 16], ep[:])
                pp2 = ps.tile([P, 264], F32, tag="projps")
                nc.tensor.matmul(pp2[:], lhsT=hcts[0][:], rhs=wt['WCC2'][0][:],
                                 start=True, stop=False)
                nc.tensor.matmul(pp2[:], lhsT=hcts[1][:], rhs=wt['WCC2'][1][:],
                                 start=False, stop=True)
                if t == 0:
                    ob_s1 = pl.tile([P, CT, 264], BF16, tag="projout1")
                    sem_state['ob'] = ob_s1
                nc.scalar.copy(sem_state['ob'][:, t, :], pp2[:])
                if t == CT - 1:
                    nc.sync.dma_start(
                        tabCC2loc.opt()[:, 0:264]
                        .rearrange("(a p) c -> p a c", p=P),
                        sem_state['ob'][:])

            for t in range(CT):
                sem_tile(0, oDC1s, oCC1s, t, sem1_consume)
            nc.gpsimd.collective_compute(
                "AllGather", AluOp.bypass,
                replica_groups=[list(range(nco))],
                ins=[tabCC2loc.opt()], outs=[tabCC2.opt()])

            # ---- L2 edge phases
            edge_phase('dc',
                       [tabDC2.opt()[0:HALF, :], tabDC2.opt()[HALF:NDP, :]],
                       lambda t: erC2s[:, t * 16:t * 16 + 8],
                       sched['dc'], make_epi_stash(3, oDC2s))
            edge_phase('cc', [tabCC2.opt()[:, :]],
                       lambda t: erC2s[:, t * 16 + 8:t * 16 + 16],
                       sched['cc'], make_epi_stash(4, oCC2s))

            # ---- sem2 + MLP head
            def sem2_consume(t, hcts):
                h1p = ps.tile([32, P], F32, tag="aux")
                nc.tensor.matmul(h1p[:], lhsT=dW1[0][:], rhs=hcts[0][:],
                                 start=True, stop=False)
                nc.tensor.matmul(h1p[:], lhsT=dW1[1][:], rhs=hcts[1][:],
                                 start=False, stop=True)
                h1 = sb.tile([32, P], BF16, tag="mlph1")
                nc.vector.scalar_tensor_tensor(
                    out=h1[:], in0=h1p[:], scalar=1.0,
                    in1=db1[:].to_broadcast([32, P]),
                    op0=AluOp.mult, op1=AluOp.add)
                nc.vector.scalar_tensor_tensor(
                    out=h1[:], in0=h1[:], scalar=0.01, in1=h1[:],
                    op0=AluOp.mult, op1=AluOp.max)
                h2p = ps.tile([16, P], F32, tag="aux")
                nc.tensor.matmul(h2p[:], lhsT=dW2[:], rhs=h1[:],
                                 start=True, stop=True)
                h2 = sb.tile([16, P], BF16, tag="mlph2")
                nc.vector.scalar_tensor_tensor(
                    out=h2[:], in0=h2p[:], scalar=1.0,
                    in1=db2[:].to_broadcast([16, P]),
                    op0=AluOp.mult, op1=AluOp.add)
                nc.vector.scalar_tensor_tensor(
                    out=h2[:], in0=h2[:], scalar=0.01, in1=h2[:],
                    op0=AluOp.mult, op1=AluOp.max)
                h3p = ps.tile([1, P], F32, tag="aux")
                nc.tensor.matmul(h3p[:], lhsT=dW3[:], rhs=h2[:],
                                 start=True, stop=True)
                h3 = sb.tile([1, P], F32, tag="mlph3")
                nc.vector.tensor_scalar(h3[:], h3p[:], db3[:], None,
                                        op0=AluOp.add)
                nc.sync.dma_start(out[0:1, t * P:(t + 1) * P], h3[:])

            for t in range(CT):
                sem_tile(1, oDC2s, oCC2s, t, sem2_consume)

    nc.compile()
    if legalize:
        legalize_waits(nc)
    return nc


# --------------------------------------------------------------------------
# entry point
# --------------------------------------------------------------------------

_CACHE = {}


def kernel(**inputs):
    cfg = make_cfg(inputs['feat_drug'].shape[0], inputs['feat_cell'].shape[0])
    sched, in_maps = host_prep(inputs, cfg)
    key = tuple(int(x) for s in sched.values() for x in s.flatten())
    if key not in _CACHE:
        _CACHE[key] = build_program(sched, cfg)
    nc = _CACHE[key]
    from concourse.bass_utils import run_bass_kernel_spmd
    res = run_bass_kernel_spmd(nc, in_maps, list(range(cfg['n_cores'])))
    pieces = [res.results[c]['out'][0] for c in range(cfg['n_cores'])]
    full = np.concatenate([p[:cfg['CBLK']] for p in pieces])[:cfg['Nc']]
    return full.reshape(-1, 1).astype(np.float32)


# revision 27
# speedup vs baseline: 1.1374x; 1.1140x over previous
"""Trainium2 Bass kernel v2 for the 2-layer heterogeneous GAT (drug/cell).

Strategy (8 NeuronCores, SPMD single program), changes vs v1:
  - All five edge phases partitioned by DST block; dc2 now dst-partitioned
    too, fed by an AllGather of hd1 (bf16) instead of AllToAll partials.
  - bf16 gather tables, 384-col rows (768B, %256B) holding [fs 256 | el 8].
  - er is never gathered: per-dst-tile er vectors [128, 8] are stashed in
    SBUF (projected from local features / hc1 tiles) and broadcast to edges
    with a one-hot matmul (lhsT = S2[dst -> edge]).
  - Edge phase batches DVE work per group of <=16 chunks: one is_equal for
    all S chunks, one for S2 (dst-row stream replicated via DMA broadcast),
    one fused exp/leaky pipeline, one rhs build.
  - Gathers up to 1024 idx per call (SWDGE ring limit is < 2048).
  - Projections batched: 8 tiles per DMA load/store, psum->SBUF copies
    alternate ACT/DVE.
"""
import sys
sys.path.insert(0, '/opt/trn_rl_repo')
import numpy as np
import ml_dtypes

import concourse.bacc as bacc
import concourse.tile as tile
from concourse import mybir, library_config

F32 = mybir.dt.float32
BF16 = mybir.dt.bfloat16
FP8 = mybir.dt.float8e4
I16 = mybir.dt.int16
BF = ml_dtypes.bfloat16
F8 = mybir.dt.np(FP8)
P = 128
H = 8
FD = 256          # feature dim
TW = 384          # gather-table row width (bf16) = 768B; payload 264
GSZ = 16          # chunks per batched edge group
GCH = 8           # chunks per dma_gather call (1024 idx; 2048 hangs)
AluOp = mybir.AluOpType
Act = mybir.ActivationFunctionType

N_CORES = 8


def legalize_waits(nc):
    """Split multi-wait instructions into wait-carrying NOP chains."""
    n_split = 0
    for fn in nc.m.functions:
        for bb in fn.blocks:
            insts = bb.instructions
            new = []
            changed = False
            for inst in insts:
                si = inst.sync_info
                waits = list(si.on_wait) if si is not None else []
                cap = 2 if isinstance(inst, mybir.InstEventSemaphore) else 1
                if len(waits) > cap:
                    keep = waits[-cap:]
                    for w in waits[:-cap]:
                        nop = mybir.InstNoOp(
                            name=nc.get_next_instruction_name(),
                            engine=inst.engine,
                            sync_info=mybir.SyncInfo(on_wait=[w], on_update=[]),
                            bass_nofuse=True,
                        )
                        new.append(nop)
                        n_split += 1
                    inst.sync_info = mybir.SyncInfo(
                        on_wait=keep, on_update=list(si.on_update))
                    changed = True
                new.append(inst)
            if changed:
                bb.instructions = new
    return n_split


def make_cfg(Nd, Nc):
    ndp = -(-Nd // 1024) * 1024
    ncp = -(-Nc // 1024) * 1024
    return dict(Nd=Nd, Nc=Nc, NDP=ndp, NCP=ncp, DBLK=ndp // 8,
                CBLK=ncp // 8, HALF=ndp // 2, n_cores=8)


# --------------------------------------------------------------------------
# host-side prep
# --------------------------------------------------------------------------

def _fold_weights(ip):
    def wel(W, a):  # W [256,256], a [H,D] -> [256,H]
        return (np.asarray(W, np.float32).reshape(FD, H, -1)
                * np.asarray(a, np.float32)[None]).sum(-1)
    Wsrc = np.asarray(ip['Wsrc'], np.float32)
    Wdst = np.asarray(ip['Wdst_dc'], np.float32)
    al = np.asarray(ip['attn_l'], np.float32)
    ar = np.asarray(ip['attn_r'], np.float32)

    def w264(W, a):
        return np.concatenate([W, wel(W, a)], 1)  # [256, 264]

    def bftile(a, ncols):  # [256, ncols] -> [2, 128, ncols] bf16
        return np.ascontiguousarray(a.reshape(2, P, ncols)).astype(BF)

    out = {}
    out['WDD'] = bftile(w264(Wsrc[0, 0], al[0, 0]), 264)
    out['WDC'] = bftile(w264(Wsrc[0, 1], al[0, 1]), 264)
    out['WCC1'] = bftile(w264(Wsrc[0, 2], al[0, 2]), 264)
    out['WDC2'] = bftile(w264(Wsrc[1, 1], al[1, 1]), 264)
    out['WCC2'] = bftile(w264(Wsrc[1, 2], al[1, 2]), 264)
    out['WerDD'] = bftile(wel(Wsrc[0, 0], ar[0, 0]), 8)
    out['WerC1'] = bftile(np.concatenate(
        [wel(Wdst[0], ar[0, 1]), wel(Wsrc[0, 2], ar[0, 2])], 1), 16)
    out['WerC2'] = bftile(np.concatenate(
        [wel(Wdst[1], ar[1, 1]), wel(Wsrc[1, 2], ar[1, 2])], 1), 16)
    out['bias5'] = np.stack([ip['gat_bias'][0, 0], ip['gat_bias'][0, 1],
                             ip['gat_bias'][0, 2], ip['gat_bias'][1, 1],
                             ip['gat_bias'][1, 2]]).astype(np.float32)
    out['semW1'] = np.stack([np.asarray(ip['sem_W1'][l, 1], np.float32)
                             .reshape(2, P, P) for l in range(2)]).astype(BF)
    out['semb1'] = np.stack([np.asarray(ip['sem_b1'][l, 1], np.float32)
                             .reshape(P, 1) for l in range(2)]).astype(np.float32)
    out['semW2'] = np.stack([np.asarray(ip['sem_W2'][l, 1], np.float32)
                             .reshape(P, 1) for l in range(2)]).astype(BF)
    out['dnnW1'] = np.asarray(ip['dnn_W1'], np.float32).reshape(2, P, 32).astype(BF)
    out['dnnb1'] = np.asarray(ip['dnn_b1'], np.float32).reshape(32, 1)
    out['dnnW2'] = np.asarray(ip['dnn_W2'], np.float32).astype(BF)
    out['dnnb2'] = np.asarray(ip['dnn_b2'], np.float32).reshape(16, 1)
    out['dnnW3'] = np.asarray(ip['dnn_W3'], np.float32).astype(BF)
    out['dnnb3'] = np.asarray(ip['dnn_b3'], np.float32).reshape(1, 1)
    return out


def _wrap16(vals):
    n = len(vals)
    out = np.zeros((P, n // 16), np.int16)
    a = np.asarray(vals, np.int16).reshape(-1, 16).T
    for g in range(8):
        out[g * 16:(g + 1) * 16, :] = a
    return out


def _prep_rel(src, dst, nco, *, blk, n_tiles, halves):
    """Per-core edge schedule, dst-partitioned.

    Returns nch [T, n_halves] (equalized over cores) and per-core streams:
    src idx wrapped i16 [P, tot*8], plus host-built one-hot scatter
    matrices in fp8: S [P, tot*128] (S[p, k*128+d] = dl[p,k]==d) and
    S2 [P, tot*128] (S2[p, k*128+e] = dl[e,k]==p).
    """
    src = np.asarray(src, np.int64)
    dst = np.asarray(dst, np.int64)
    per_core = []
    for c in range(nco):
        m = (dst >= c * blk) & (dst < (c + 1) * blk)
        dl = dst[m] - c * blk
        s = src[m]
        tid = dl // P
        per_core.append((s, dl % P, tid))
    nh = 2 if halves else 1
    nch = np.zeros((n_tiles, nh), np.int64)
    buckets = []
    for c in range(nco):
        s, dloc, tid = per_core[c]
        bk = {}
        for t in range(n_tiles):
            mt = tid == t
            st, dt_ = s[mt], dloc[mt]
            if halves:
                m0 = st < halves
                groups = [(st[m0], dt_[m0]), (st[~m0] - halves, dt_[~m0])]
            else:
                groups = [(st, dt_)]
            bk[t] = groups
            for h, (gs, gd) in enumerate(groups):
                nch[t, h] = max(nch[t, h], (len(gs) + P - 1) // P)
        buckets.append(bk)
    nch = np.maximum(nch, 1)
    tot = int(nch.sum())
    rng = np.arange(P, dtype=np.int64)
    srcs, Ss, S2s = [], [], []
    for c in range(nco):
        bk = buckets[c]
        sw = np.zeros((P, tot * 8), np.int16)
        dall = np.full((tot, P), -1, np.int64)   # [chunk, edge] local dst
        col = 0
        for t in range(n_tiles):
            for h, (gs, gd) in enumerate(bk[t]):
                n = int(nch[t, h]) * P
                a = np.zeros(n, np.int64)
                a[:len(gs)] = gs
                assert a.max(initial=0) < 32768
                sw[:, col * 8:(col + int(nch[t, h])) * 8] = _wrap16(a)
                dpad = np.full(n, -1, np.int64)
                dpad[:len(gd)] = gd
                dall[col:col + int(nch[t, h])] = dpad.reshape(-1, P)
                col += int(nch[t, h])
        # S[p, k, d] = (dall[k, p] == d);  S2[p, k, e] = (dall[k, e] == p)
        S = (dall.T[:, :, None] == rng[None, None, :]).astype(F8)
        S2 = (dall[None, :, :] == rng[:, None, None]).astype(F8)
        srcs.append(sw)
        Ss.append(np.ascontiguousarray(S.reshape(P, tot * P)))
        S2s.append(np.ascontiguousarray(S2.reshape(P, tot * P)))
    return dict(nch=nch, src=srcs, S=Ss, S2=S2s, tot=tot)


def host_prep(ip, cfg):
    W = _fold_weights(ip)
    nco = cfg['n_cores']
    DBLK, CBLK, HALF = cfg['DBLK'], cfg['CBLK'], cfg['HALF']
    NDP, NCP = cfg['NDP'], cfg['NCP']
    DD_T, CT = DBLK // P, CBLK // P

    dd = _prep_rel(ip['src_dd'], ip['dst_dd'], nco, blk=DBLK,
                   n_tiles=DD_T, halves=HALF)
    dc = _prep_rel(ip['src_dc'], ip['dst_dc'], nco, blk=CBLK,
                   n_tiles=CT, halves=HALF)
    cc = _prep_rel(ip['src_cc'], ip['dst_cc'], nco, blk=CBLK,
                   n_tiles=CT, halves=None)

    featD = np.zeros((NDP, FD), np.float32)
    featD[:cfg['Nd']] = np.asarray(ip['feat_drug'], np.float32)
    featC = np.zeros((NCP, FD), np.float32)
    featC[:cfg['Nc']] = np.asarray(ip['feat_cell'], np.float32)
    featDT = np.ascontiguousarray(featD.T).reshape(2, P, NDP).astype(BF)
    featCT = np.ascontiguousarray(featC.T).reshape(2, P, NCP).astype(BF)

    identB = np.eye(P, dtype=np.float32).astype(BF)

    base = dict(featDT=featDT, featCT=featCT, identB=identB, **W)
    in_maps = []
    for c in range(nco):
        m = dict(base)
        m['featDTloc'] = np.ascontiguousarray(
            featD[c * DBLK:(c + 1) * DBLK].T).reshape(2, P, DBLK).astype(BF)
        m['featCTloc'] = np.ascontiguousarray(
            featC[c * CBLK:(c + 1) * CBLK].T).reshape(2, P, CBLK).astype(BF)
        for r, d in (('dd', dd), ('dc', dc), ('cc', cc)):
            m[f'{r}_src'] = d['src'][c]
            m[f'{r}_S'] = d['S'][c]
            m[f'{r}_S2'] = d['S2'][c]
        in_maps.append(m)
    sched = dict(dd=dd['nch'], dc=dc['nch'], cc=cc['nch'])
    return sched, in_maps


# --------------------------------------------------------------------------
# device program
# --------------------------------------------------------------------------

def _tile_segments(nch_row):
    """Per-tile gather segments [(start_col, n, half)], respecting half
    boundaries, GSZ group windows, and GCH call caps."""
    bounds = []
    off = 0
    for h, nh in enumerate(nch_row):
        bounds.append((off, off + int(nh), h))
        off += int(nh)
    tnch = off
    segs = []
    for g0 in range(0, tnch, GSZ):
        g1 = min(g0 + GSZ, tnch)
        for (h0, h1, h) in bounds:
            s0 = max(g0, h0)
            s1 = min(g1, h1)
            for b in range(s0, s1, GCH):
                segs.append((b, min(GCH, s1 - b), h))
    return segs, tnch


def build_program(sched, cfg, legalize=True):
    nco = cfg['n_cores']
    DBLK, CBLK, HALF = cfg['DBLK'], cfg['CBLK'], cfg['HALF']
    NDP, NCP = cfg['NDP'], cfg['NCP']
    DD_T, CT = DBLK // P, CBLK // P
    NDT, NCT = NDP // P, NCP // P

    nc = bacc.Bacc(None)
    d = {}
    def inp(name, shape, dt=BF16):
        d[name] = nc.declare_dram_parameter(name, list(shape), dt,
                                            isOutput=False)
        return d[name]

    featDT = inp('featDT', (2, P, NDP))
    featCT = inp('featCT', (2, P, NCP))
    featDTloc = inp('featDTloc', (2, P, DBLK))
    featCTloc = inp('featCTloc', (2, P, CBLK))
    identB_in = inp('identB', (P, P))
    Wmain = {k: inp(k, (2, P, 264)) for k in
             ('WDD', 'WDC', 'WCC1', 'WDC2', 'WCC2')}
    WerDD = inp('WerDD', (2, P, 8))
    WerC1 = inp('WerC1', (2, P, 16))
    WerC2 = inp('WerC2', (2, P, 16))
    bias5 = inp('bias5', (5, FD), F32)
    semW1 = inp('semW1', (2, 2, P, P))
    semb1 = inp('semb1', (2, P, 1), F32)
    semW2 = inp('semW2', (2, P, 1))
    dnnW1 = inp('dnnW1', (2, P, 32)); dnnb1 = inp('dnnb1', (32, 1), F32)
    dnnW2 = inp('dnnW2', (32, 16)); dnnb2 = inp('dnnb2', (16, 1), F32)
    dnnW3 = inp('dnnW3', (16, 1)); dnnb3 = inp('dnnb3', (1, 1), F32)
    streams = {}
    for r, nchs in sched.items():
        tot = int(nchs.sum())
        streams[r] = (inp(f'{r}_src', (P, tot * 8), I16),
                      inp(f'{r}_S', (P, tot * P), FP8),
                      inp(f'{r}_S2', (P, tot * P), FP8))
    out = nc.declare_dram_parameter('out', [1, CBLK], F32, isOutput=True)

    with tile.TileContext(nc) as tc:
        with tc.tile_pool(name="const", bufs=1) as cpool, \
             tc.tile_pool(name="sb", bufs=3) as sb, \
             tc.tile_pool(name="pl", bufs=3) as pl, \
             tc.tile_pool(name="gb", bufs=3) as gb, \
             tc.tile_pool(name="eb", bufs=3) as eb, \
             tc.tile_pool(name="ps", bufs=2, space="PSUM") as ps, \
             tc.tile_pool(name="psm", bufs=2, space="PSUM") as psm, \
             tc.tile_pool(name="dram", bufs=1, space="DRAM") as dr:
            nc.gpsimd.load_library(library_config.mlp)

            # ---- constants in SBUF
            C = {}
            def cload(name, ap, shape, dt=BF16):
                t_ = cpool.tile(list(shape), dt, tag=f"c_{name}")
                nc.sync.dma_start(t_[:], ap)
                return t_
            C['ident'] = cload('ident', identB_in[:], (P, P))
            ones1 = cpool.tile([1, P], BF16)
            nc.vector.memset(ones1[:], 1.0)
            wt = {}
            for k in Wmain:
                wt[k] = tuple(cload(f'{k}{j}', Wmain[k][j], (P, 264))
                              for j in range(2))
            werDD = tuple(cload(f'werDD{j}', WerDD[j], (P, 8))
                          for j in range(2))
            werC1 = tuple(cload(f'werC1{j}', WerC1[j], (P, 16))
                          for j in range(2))
            werC2 = tuple(cload(f'werC2{j}', WerC2[j], (P, 16))
                          for j in range(2))
            btiles = [cload(f'bias{r}', bias5[r:r + 1, :].to_broadcast([P, FD]),
                            (P, FD), F32) for r in range(5)]
            sw1 = [tuple(cload(f'sw1_{l}{j}', semW1[l, j], (P, P))
                         for j in range(2)) for l in range(2)]
            sb1 = [cload(f'sb1_{l}', semb1[l], (P, 1), F32) for l in range(2)]
            sw2 = [cload(f'sw2_{l}', semW2[l], (P, 1)) for l in range(2)]
            dW1 = tuple(cload(f'dW1{j}', dnnW1[j], (P, 32)) for j in range(2))
            dW2 = cload('dW2', dnnW2[:], (32, 16))
            dW3 = cload('dW3', dnnW3[:], (16, 1))
            db1 = cload('db1', dnnb1[:], (32, 1), F32)
            db2 = cload('db2', dnnb2[:], (16, 1), F32)
            db3 = cload('db3', dnnb3[:], (1, 1), F32)
            # resident local features (for er projections)
            fdl = tuple(cload(f'fdl{j}', featDTloc[j], (P, DBLK))
                        for j in range(2))
            fcl = tuple(cload(f'fcl{j}', featCTloc[j], (P, CBLK))
                        for j in range(2))
            # resident edge streams
            SI = {}
            for r in sched:
                tot = int(sched[r].sum())
                SI[r] = cload(f'si_{r}', streams[r][0][:], (P, tot * 8), I16)
            # er stashes
            erDDs = cpool.tile([P, DD_T * 8], BF16, tag="erDDs")
            erC1s = cpool.tile([P, CT * 16], BF16, tag="erC1s")
            erC2s = cpool.tile([P, CT * 16], BF16, tag="erC2s")
            # transposed per-relation output stashes [P, tile, kchunk, node]
            oDC1s = cpool.tile([P, CT, 2, P], BF16, tag="oDC1s")
            oCC1s = cpool.tile([P, CT, 2, P], BF16, tag="oCC1s")
            oDC2s = cpool.tile([P, CT, 2, P], BF16, tag="oDC2s")
            oCC2s = cpool.tile([P, CT, 2, P], BF16, tag="oCC2s")

            # ---- internal DRAM
            tabDD = dr.tile([NDP, TW], BF16)
            tabDC = dr.tile([NDP, TW], BF16)
            tabCC1 = dr.tile([NCP, TW], BF16)
            tabDC2loc = dr.tile([DBLK, TW], BF16)
            tabDC2 = dr.tile([NDP, TW], BF16, addr_space="Shared")
            tabCC2loc = dr.tile([CBLK, TW], BF16)
            tabCC2 = dr.tile([NCP, TW], BF16, addr_space="Shared")

            # ---- er stash projections (local features)
            def er_stash(fres, wpair, stash, n_tiles, ncols):
                BT = min(8, n_tiles) if ncols == 8 else n_tiles
                for t0 in range(0, n_tiles, BT):
                    bt = min(BT, n_tiles - t0)
                    pp = ps.tile([P, bt * ncols], F32, tag="projps")
                    for i in range(bt):
                        tl = (t0 + i) * P
                        nc.tensor.matmul(pp[:, i * ncols:(i + 1) * ncols],
                                         lhsT=fres[0][:, tl:tl + P],
                                         rhs=wpair[0][:], start=True, stop=False)
                        nc.tensor.matmul(pp[:, i * ncols:(i + 1) * ncols],
                                         lhsT=fres[1][:, tl:tl + P],
                                         rhs=wpair[1][:], start=False, stop=True)
                    nc.scalar.copy(stash[:, t0 * ncols:(t0 + bt) * ncols], pp[:])
            er_stash(fdl, werDD, erDDs, DD_T, 8)
            er_stash(fcl, werC1, erC1s, CT, 16)

            # ---- batched projection pass (multi-job: shared lhs loads)
            def proj(lhs_ap_fn, n_tiles, jobs, BT=8):
                """lhs_ap_fn(k, c0, n) -> DRAM AP [P, n] for k-chunk cols.
                jobs: list of (wpair, tab)."""
                for t0 in range(0, n_tiles, BT):
                    bt = min(BT, n_tiles - t0)
                    lh = pl.tile([P, 2, bt * P], BF16, tag="projlh")
                    nc.sync.dma_start(lh[:, 0, :], lhs_ap_fn(0, t0 * P, bt * P))
                    nc.sync.dma_start(lh[:, 1, :], lhs_ap_fn(1, t0 * P, bt * P))
                    for j, (wpair, tab) in enumerate(jobs):
                        ob = pl.tile([P, bt, 264], BF16, tag=f"projout{j}")
                        for i in range(bt):
                            pp = ps.tile([P, 264], F32, tag="projps")
                            nc.tensor.matmul(pp[:],
                                             lhsT=lh[:, 0, i * P:(i + 1) * P],
                                             rhs=wpair[0][:],
                                             start=True, stop=False)
                            nc.tensor.matmul(pp[:],
                                             lhsT=lh[:, 1, i * P:(i + 1) * P],
                                             rhs=wpair[1][:],
                                             start=False, stop=True)
                            nc.scalar.copy(ob[:, i, :], pp[:])
                        nc.sync.dma_start(
                            tab[t0 * P:(t0 + bt) * P, 0:264]
                            .rearrange("(t p) c -> p t c", p=P), ob[:])

            def dram_lhs(apx):
                return lambda k, c0, n: apx[k, :, c0:c0 + n]

            # L1 projections: tabDD+tabDC share feature loads
            proj(dram_lhs(featDT), NDT, [(wt['WDD'], tabDD.opt()),
                                         (wt['WDC'], tabDC.opt())])
            proj(dram_lhs(featCT), NCT, [(wt['WCC1'], tabCC1.opt())])

            # ---- edge phase
            def edge_phase(rel, tab_halves, er_fn, nchs, epilogue):
                si = SI[rel]
                S_dram, S2_dram = streams[rel][1], streams[rel][2]
                T = nchs.shape[0]
                col = 0
                for t in range(T):
                    segs, tnch = _tile_segments(nchs[t])
                    mps = psm.tile([P, 264], F32, tag="mainps")
                    er_t = er_fn(t)
                    for g0 in range(0, tnch, GSZ):
                        gn = min(GSZ, tnch - g0)
                        G = gb.tile([P, gn, TW], BF16, tag="G")
                        for (s0, sn, h) in segs:
                            if s0 < g0 or s0 >= g0 + gn:
                                continue
                            nidx = sn * P
                            nc.gpsimd.dma_gather(
                                G[:, s0 - g0:s0 - g0 + sn, :], tab_halves[h],
                                si[:, (col + s0) * 8:(col + s0 + sn) * 8],
                                nidx, nidx, TW)
                        S = eb.tile([P, gn * P], FP8, tag="S")
                        nc.sync.dma_start(
                            S[:], S_dram[:, (col + g0) * P:(col + g0 + gn) * P])
                        S2 = eb.tile([P, gn * P], FP8, tag="S2")
                        nc.sync.dma_start(
                            S2[:], S2_dram[:, (col + g0) * P:(col + g0 + gn) * P])
                        erps = ps.tile([P, gn * 8], F32, tag="erps")
                        for k in range(gn):
                            nc.tensor.matmul(
                                erps[:, k * 8:(k + 1) * 8],
                                lhsT=S2[:, k * P:(k + 1) * P], rhs=er_t,
                                start=True, stop=True, skip_group_check=True)
                        ww = eb.tile([P, gn, 8], F32, tag="ww")
                        nc.vector.tensor_tensor(
                            out=ww[:], in0=G[:, :, FD:FD + 8],
                            in1=erps[:].rearrange("p (a b) -> p a b", b=8),
                            op=AluOp.add)
                        nc.vector.scalar_tensor_tensor(
                            out=ww[:], in0=ww[:], scalar=0.2, in1=ww[:],
                            op0=AluOp.mult, op1=AluOp.max)
                        nc.scalar.activation(ww[:], ww[:], Act.Exp)
                        rhs = eb.tile([P, gn, 264], BF16, tag="rhs")
                        nc.vector.tensor_tensor(
                            out=rhs[:, :, 0:FD]
                                .rearrange("p a (h e) -> p a h e", h=H),
                            in0=G[:, :, 0:FD]
                                .rearrange("p a (h e) -> p a h e", h=H),
                            in1=ww[:, :, :, None].to_broadcast([P, gn, H, 32]),
                            op=AluOp.mult)
                        nc.vector.tensor_copy(rhs[:, :, FD:FD + 8], ww[:])
                        for k in range(gn):
                            nc.tensor.matmul(
                                mps[:], lhsT=S[:, k * P:(k + 1) * P],
                                rhs=rhs[:, k, :],
                                start=(g0 == 0 and k == 0),
                                stop=(g0 + gn == tnch and k == gn - 1),
                                skip_group_check=True)
                    epilogue(t, mps)
                    col += tnch

            def _norm_elu(pp, bias_tile, o_bf):
                den = sb.tile([P, 8], F32, tag="den")
                nc.vector.tensor_scalar_max(den[:], pp[:, FD:FD + 8], 1e-30)
                rec = sb.tile([P, 8], F32, tag="rec")
                nc.vector.reciprocal(rec[:], den[:])
                x = sb.tile([P, FD], F32, tag="xnrm")
                nc.vector.tensor_tensor(
                    out=x[:].rearrange("p (h e) -> p h e", h=H),
                    in0=pp[:, 0:FD].rearrange("p (h e) -> p h e", h=H),
                    in1=rec[:, :, None].to_broadcast([P, H, 32]),
                    op=AluOp.mult)
                nc.vector.tensor_add(x[:], x[:], bias_tile[:])
                ex = sb.tile([P, FD], F32, tag="eluex")
                nc.scalar.activation(ex[:], x[:], Act.Exp)
                nc.scalar.activation(ex[:], ex[:], Act.Relu, bias=1.0, scale=-1.0)
                xp = sb.tile([P, FD], F32, tag="elup")
                nc.scalar.activation(xp[:], x[:], Act.Relu)
                nc.vector.tensor_sub(o_bf[:], xp[:], ex[:])

            def _transp(o_bf, kk):
                tp = ps.tile([P, P], BF16, space="PSUM", tag="aux")
                nc.tensor.transpose(tp[:], o_bf[:, kk * P:(kk + 1) * P],
                                    C['ident'][:])
                return tp

            def make_epi_stash(bias_idx, stash):
                def epi(t, mps):
                    o = sb.tile([P, FD], BF16, tag="oed")
                    _norm_elu(mps, btiles[bias_idx], o)
                    for kk in range(2):
                        tp = _transp(o, kk)
                        nc.scalar.copy(stash[:, t, kk, :], tp[:])
                return epi

            # dd epilogue fuses the tabDC2 local-shard projection: the
            # transposed hd1 tile is exactly the proj lhsT, so hd1 never
            # round-trips through DRAM.
            ddep_state = {}
            def dd_epi(t, mps, T=DD_T):
                rs = min(8, T)
                o = sb.tile([P, FD], BF16, tag="oed")
                _norm_elu(mps, btiles[0], o)
                ts = sb.tile([P, 2, P], BF16, tag="tpsb")
                for kk in range(2):
                    tp = _transp(o, kk)
                    nc.scalar.copy(ts[:, kk, :], tp[:])
                pp2 = ps.tile([P, 264], F32, tag="projps")
                nc.tensor.matmul(pp2[:], lhsT=ts[:, 0, :], rhs=wt['WDC2'][0][:],
                                 start=True, stop=False)
                nc.tensor.matmul(pp2[:], lhsT=ts[:, 1, :], rhs=wt['WDC2'][1][:],
                                 start=False, stop=True)
                if t % rs == 0:
                    ob_dd = pl.tile([P, rs, 264], BF16, tag="projout0")
                    ddep_state['ob'] = ob_dd
                nc.scalar.copy(ddep_state['ob'][:, t % rs, :], pp2[:])
                if t % rs == rs - 1 or t == T - 1:
                    cnt = t % rs + 1
                    t0 = t - t % rs
                    nc.sync.dma_start(
                        tabDC2loc.opt()[t0 * P:(t + 1) * P, 0:264]
                        .rearrange("(a p) c -> p a c", p=P),
                        ddep_state['ob'][:, 0:cnt, :])

            # ---- L1 edge phases
            edge_phase('dd',
                       [tabDD.opt()[0:HALF, :], tabDD.opt()[HALF:NDP, :]],
                       lambda t: erDDs[:, t * 8:(t + 1) * 8],
                       sched['dd'], dd_epi)
            nc.gpsimd.collective_compute(
                "AllGather", AluOp.bypass,
                replica_groups=[list(range(nco))],
                ins=[tabDC2loc.opt()], outs=[tabDC2.opt()])
            edge_phase('dc',
                       [tabDC.opt()[0:HALF, :], tabDC.opt()[HALF:NDP, :]],
                       lambda t: erC1s[:, t * 16:t * 16 + 8],
                       sched['dc'], make_epi_stash(1, oDC1s))
            edge_phase('cc', [tabCC1.opt()[:, :]],
                       lambda t: erC1s[:, t * 16 + 8:t * 16 + 16],
                       sched['cc'], make_epi_stash(2, oCC1s))

            # ---- semantic attention (cells) + optional er stash / head
            def sem_tile(l, oDCs_, oCCs_, t, consume):
                z = [oDCs_[:, t, :, :], oCCs_[:, t, :, :]]
                wms = []
                for m in range(2):
                    hp = ps.tile([P, P], F32, tag="aux")
                    nc.tensor.matmul(hp[:], lhsT=sw1[l][0][:], rhs=z[m][:, 0, :],
                                     start=True, stop=False)
                    nc.tensor.matmul(hp[:], lhsT=sw1[l][1][:], rhs=z[m][:, 1, :],
                                     start=False, stop=True)
                    ht = sb.tile([P, P], BF16, tag="semh")
                    nc.scalar.activation(ht[:], hp[:], Act.Tanh, bias=sb1[l][:])
                    wp = ps.tile([1, P], F32, tag="aux")
                    nc.tensor.matmul(wp[:], lhsT=sw2[l][:], rhs=ht[:],
                                     start=True, stop=True)
                    wm = sb.tile([1, P], F32, tag="semw")
                    nc.scalar.copy(wm[:], wp[:])
                    wms.append(wm)
                beta = sb.tile([1, P], BF16, tag="semb")
                nc.vector.tensor_sub(beta[:], wms[0][:], wms[1][:])
                nc.scalar.activation(beta[:], beta[:], Act.Sigmoid)
                bb = ps.tile([P, P], F32, tag="aux")
                nc.tensor.matmul(bb[:], lhsT=ones1[:], rhs=beta[:],
                                 start=True, stop=True)
                hcts = []
                for kk in range(2):
                    diff = sb.tile([P, P], BF16, tag="semd")
                    nc.vector.tensor_sub(diff[:], z[0][:, kk, :], z[1][:, kk, :])
                    nc.vector.tensor_mul(diff[:], diff[:], bb[:])
                    hct = sb.tile([P, P], BF16, tag="semhc")
                    nc.vector.tensor_add(hct[:], z[1][:, kk, :], diff[:])
                    hcts.append(hct)
                consume(t, hcts)

            # sem1 fuses the tabCC2 local-shard projection (hct is the lhsT)
            sem_state = {}
            def sem1_consume(t, hcts):
                ep = ps.tile([P, 16], F32, tag="aux")
                nc.tensor.matmul(ep[:], lhsT=hcts[0][:], rhs=werC2[0][:],
                                 start=True, stop=False)
                nc.tensor.matmul(ep[:], lhsT=hcts[1][:], rhs=werC2[1][:],
                                 start=False, stop=True)
                nc.scalar.copy(erC2s[:, t * 16:(t + 1) * 16], ep[:])
                pp2 = ps.tile([P, 264], F32, tag="projps")
                nc.tensor.matmul(pp2[:], lhsT=hcts[0][:], rhs=wt['WCC2'][0][:],
                                 start=True, stop=False)
                nc.tensor.matmul(pp2[:], lhsT=hcts[1][:], rhs=wt['WCC2'][1][:],
                                 start=False, stop=True)
                if t == 0:
                    ob_s1 = pl.tile([P, CT, 264], BF16, tag="projout1")
                    sem_state['ob'] = ob_s1
                nc.scalar.copy(sem_state['ob'][:, t, :], pp2[:])
                if t == CT - 1:
                    nc.sync.dma_start(
                        tabCC2loc.opt()[:, 0:264]
                        .rearrange("(a p) c -> p a c", p=P),
                        sem_state['ob'][:])

            for t in range(CT):
                sem_tile(0, oDC1s, oCC1s, t, sem1_consume)
            nc.gpsimd.collective_compute(
                "AllGather", AluOp.bypass,
                replica_groups=[list(range(nco))],
                ins=[tabCC2loc.opt()], outs=[tabCC2.opt()])

            # ---- L2 edge phases
            edge_phase('dc',
                       [tabDC2.opt()[0:HALF, :], tabDC2.opt()[HALF:NDP, :]],
                       lambda t: erC2s[:, t * 16:t * 16 + 8],
                       sched['dc'], make_epi_stash(3, oDC2s))
            edge_phase('cc', [tabCC2.opt()[:, :]],
                       lambda t: erC2s[:, t * 16 + 8:t * 16 + 16],
                       sched['cc'], make_epi_stash(4, oCC2s))

            # ---- sem2 + MLP head
            def sem2_consume(t, hcts):
                h1p = ps.tile([32, P], F32, tag="aux")
                nc.tensor.matmul(h1p[:], lhsT=dW1[0][:], rhs=hcts[0][:],
                                 start=True, stop=False)
                nc.tensor.matmul(h1p[:], lhsT=dW1[1][:], rhs=hcts[1][:],
                                 start=False, stop=True)
                h1 = sb.tile([32, P], BF16, tag="mlph1")
                nc.vector.scalar_tensor_tensor(
                    out=h1[:], in0=h1p[:], scalar=1.0,
                    in1=db1[:].to_broadcast([32, P]),
                    op0=AluOp.mult, op1=AluOp.add)
                nc.vector.scalar_tensor_tensor(
                    out=h1[:], in0=h1[:], scalar=0.01, in1=h1[:],
                    op0=AluOp.mult, op1=AluOp.max)
                h2p = ps.tile([16, P], F32, tag="aux")
                nc.tensor.matmul(h2p[:], lhsT=dW2[:], rhs=h1[:],
                                 start=True, stop=True)
                h2 = sb.tile([16, P], BF16, tag="mlph2")
                nc.vector.scalar_tensor_tensor(
                    out=h2[:], in0=h2p[:], scalar=1.0,
                    in1=db2[:].to_broadcast([16, P]),
                    op0=AluOp.mult, op1=AluOp.add)
                nc.vector.scalar_tensor_tensor(
                    out=h2[:], in0=h2[:], scalar=0.01, in1=h2[:],
                    op0=AluOp.mult, op1=AluOp.max)
                h3p = ps.tile([1, P], F32, tag="aux")
                nc.tensor.matmul(h3p[:], lhsT=dW3[:], rhs=h2[:],
                                 start=True, stop=True)
                h3 = sb.tile([1, P], F32, tag="mlph3")
                nc.vector.tensor_scalar(h3[:], h3p[:], db3[:], None,
                                        op0=AluOp.add)
                nc.sync.dma_start(out[0:1, t * P:(t + 1) * P], h3[:])

            for t in range(CT):
                sem_tile(1, oDC2s, oCC2s, t, sem2_consume)

    nc.compile()
    if legalize:
        legalize_waits(nc)
    return nc


# --------------------------------------------------------------------------
# entry point
# --------------------------------------------------------------------------

_CACHE = {}


def kernel(**inputs):
    cfg = make_cfg(inputs['feat_drug'].shape[0], inputs['feat_cell'].shape[0])
    sched, in_maps = host_prep(inputs, cfg)
    key = tuple(int(x) for s in sched.values() for x in s.flatten())
    if key not in _CACHE:
        _CACHE[key] = build_program(sched, cfg)
    nc = _CACHE[key]
    from concourse.bass_utils import run_bass_kernel_spmd
    res = run_bass_kernel_spmd(nc, in_maps, list(range(cfg['n_cores'])))
    pieces = [res.results[c]['out'][0] for c in range(cfg['n_cores'])]
    full = np.concatenate([p[:cfg['CBLK']] for p in pieces])[:cfg['Nc']]
    return full.reshape(-1, 1).astype(np.float32)
